# revision 4
# baseline (speedup 1.0000x reference)
"""AttentionDTI forward pass on 8 TRN2 NeuronCores — pure data parallel over batch.

Model (B=8, LD=100, LP=1000, DIM=64, CONV=40, C4=160):
  embed -> 3x conv1d+relu (drug: k=4,6,8 ; protein: k=4,8,12)
  d_att = dc^T @ d_att_w + b ; p_att = pc^T @ p_att_w + b
  R = relu(d_att[:,i,None,:] + p_att[:,None,j,:])      # [B,85,979,160] never materialized
  comp_atte = sigmoid((R.mean(2) @ att_w + att_b)^T)   # via S[c,i] = sum_j relu(...)
  prot_atte = sigmoid((R.mean(1) @ att_w + att_b)^T)   # via T[c,j] = sum_i relu(...)
  gate, global max pool, FC 320->1024->1024->512->2 (leaky relu 0.01)

Sharding: core b handles batch element b. All params replicated. No collectives.

v2 changes vs baseline:
- All parameters packed host-side into a few [128, W] DRAM blobs, loaded with
  single large DMAs (the baseline's 105 small DMAs serialized ~60us on the SP
  sequencer at 565ns each and kept the DMA queue in tiny <2KB packets).
  Input indices go first; FC blob rides the second HWDGE queue (Activation).
- R loop: DVE iterations use tensor_scalar (TensorScalarPtr supports the DVE
  4x perf mode for packed bf16 SBUF operands) instead of scalar_tensor_tensor
  (no perf modes). ACT iterations keep activation+accum. Additionally DVE
  folds some pairs of tmp tiles (tensor_tensor add, 2x mode) so the PE
  identity-matmul T-accumulation streams fewer tiles.
- PE warmup decoupled from DMA (gpsimd memset source) and shortened.
"""

import numpy as np

B, LD, LP, DIM, CONV = 8, 100, 1000, 64, 40
C4 = 160
LD1, LD2, LD3 = 97, 92, 85     # drug conv output lengths (k=4,6,8)
LP1, LP2, LP3 = 997, 990, 979  # protein conv output lengths (k=4,8,12)
NB = 22                        # ceil(85/4) packed iterations for chunk B

# tuning knobs
WARMUP_MM = 6
NACT_A, NFOLD_A = 26, 20       # chunk-A (85 iters): #on ACT engine, #DVE pair-folds
NACT_B, NFOLD_B = 7, 5         # chunk-B (22 iters)

_CACHE = {}

# ---------------- blob layouts (shared by build and host prep) ----------------
# each entry: (name, rows, cols)
L_M = ([("ones", 1, 128), ("embd", 65, DIM), ("embp", 26, DIM)]
       + [(f"dw1_{k}", DIM, CONV) for k in range(4)]
       + [(f"pw1_{k}", DIM, CONV) for k in range(4)])
L_E = ([(f"pw2_{k}", CONV, 2 * CONV) for k in range(8)]
       + [(f"pw3_{k}", 2 * CONV, C4) for k in range(12)]
       + [(f"dw2_{k}", CONV, 2 * CONV) for k in range(6)]
       + [(f"dw3_{k}", 2 * CONV, C4) for k in range(8)])
L_A = ([("id128", 128, 128), ("id4", 128, 32)]
       + [(f"{w}{c}", 128 if c == "A" else 32, C4)
          for w in ("daw", "paw", "aw") for c in ("A", "B")]
       + [(f"{w}{c}", 128 if c == "A" else 32, 128)
          for w in ("dawr", "pawr") for c in ("A", "B")])
FCCH = [(0, 128), (128, 32), (160, 128), (288, 32)]
L_FC = ([(f"fc1w_{i}", w, 1024) for i, (o, w) in enumerate(FCCH)]
        + [(f"fc2w_{g}", 128, 1024) for g in range(8)]
        + [(f"fc3w_{g}", 128, 512) for g in range(8)]
        + [(f"outw_{g}", 128, 2) for g in range(4)])
L_F = ([("iota", 128, 1), ("db1", CONV, 1), ("db2", 2 * CONV, 1),
        ("db3A", 128, 1), ("db3B", 32, 1), ("pb1", CONV, 1),
        ("pb2", 2 * CONV, 1), ("pb3A", 128, 1), ("pb3B", 32, 1),
        ("dabA", 128, 1), ("dabB", 32, 1), ("pabA", 128, 1), ("pabB", 32, 1),
        ("abA", 128, 1), ("abB", 32, 1), ("dabr", 128, 1), ("pabr", 128, 1),
        ("fc1b", 128, 8), ("fc2b", 128, 8), ("fc3b", 128, 4), ("outb", 2, 1)])


def _layout(items):
    pos, c = {}, 0
    for name, r, w in items:
        pos[name] = (r, c, w)
        c += w
    return pos, c


POS_M, W_M = _layout(L_M)
POS_E, W_E = _layout(L_E)
POS_A, W_A = _layout(L_A)
POS_FC, W_FC = _layout(L_FC)
POS_F, W_F = _layout(L_F)
BLOBS = [("blob_m", POS_M, W_M), ("blob_e", POS_E, W_E), ("blob_a", POS_A, W_A),
         ("blob_fc", POS_FC, W_FC), ("blob_f", POS_F, W_F)]


def _build():
    from contextlib import ExitStack
    import concourse.bass as bass
    import concourse.tile as tile
    from concourse import bacc, mybir

    f32 = mybir.dt.float32
    bf16 = mybir.dt.bfloat16
    AF = mybir.ActivationFunctionType
    ALU = mybir.AluOpType
    AX = mybir.AxisListType

    nc = bacc.Bacc("TRN2", target_bir_lowering=False, debug=False)

    d_idx = nc.declare_dram_parameter("drug_idx", [1, LD], bf16, isOutput=False)
    p_idx = nc.declare_dram_parameter("prot_idx", [1, LP], bf16, isOutput=False)
    blob_d = {}
    for bname, pos, w in BLOBS:
        dt = f32 if bname == "blob_f" else bf16
        blob_d[bname] = nc.declare_dram_parameter(bname, [128, w], dt, isOutput=False)
    out_d = nc.declare_dram_parameter("out", [2, 1], f32, isOutput=True)

    CH = [(0, 128), (128, 32)]  # (offset, width) chunks of the 160 dim

    with tile.TileContext(nc) as tc, ExitStack() as ctx:
        wp = ctx.enter_context(tc.tile_pool(name="w", bufs=1))
        ap_ = ctx.enter_context(tc.tile_pool(name="a", bufs=1))
        tp = ctx.enter_context(tc.tile_pool(name="t", bufs=8))
        fp = ctx.enter_context(tc.tile_pool(name="f", bufs=4))
        pp = ctx.enter_context(tc.tile_pool(name="p", bufs=2, space="PSUM"))
        pT = ctx.enter_context(tc.tile_pool(name="pT", bufs=1, space="PSUM"))

        # ---- blob DMAs; indices first; FC blob on the second HWDGE queue ----
        nc_blob = {}
        idx_d_t = ap_.tile([1, LD], bf16, tag="idx_d")
        nc.sync.dma_start(out=idx_d_t[:], in_=d_idx[:])
        idx_p_t = ap_.tile([1, LP], bf16, tag="idx_p")
        nc.sync.dma_start(out=idx_p_t[:], in_=p_idx[:])
        for bname, pos, w in BLOBS:
            dt = f32 if bname == "blob_f" else bf16
            t = wp.tile([128, w], dt, tag=bname)
            if bname == "blob_fc":
                nc.scalar.dma_start(out=t[:], in_=blob_d[bname][:])
            else:
                nc.sync.dma_start(out=t[:], in_=blob_d[bname][:])
            nc_blob[bname] = t

        def g(bname, name, rs=None, cs=None):
            """AP for packed tile `name` in blob `bname`, optionally sub-sliced."""
            r, c0, w = {"blob_m": POS_M, "blob_e": POS_E, "blob_a": POS_A,
                        "blob_fc": POS_FC, "blob_f": POS_F}[bname][name]
            r0, r1 = (0, r) if rs is None else (rs[0], rs[1])
            co, cw = (0, w) if cs is None else cs
            return nc_blob[bname][r0:r1, c0 + co:c0 + co + cw]

        # ---- PE warmup while DMAs land: memset source, no DMA dependency ----
        wu = ap_.tile([128, 512], bf16, tag="wu")
        nc.gpsimd.memset(wu[:], 0.0)
        ps_wu = pT.tile([128, 512], mybir.dt.float32, tag="wu")
        for _ in range(WARMUP_MM):
            nc.tensor.matmul(ps_wu[:], wu[:, 0:128], wu[:], start=True, stop=True)

        # ---- one-hot + embedding ----
        def embed(idx_t, nvocab, L, emb_ap, tag):
            e = ap_.tile([DIM, L], bf16, tag=f"e_{tag}")
            for l0 in range(0, L, 512):
                cs = min(512, L - l0)
                psb = pp.tile([nvocab, 512], f32, tag="ps")
                nc.tensor.matmul(psb[:, :cs], g("blob_m", "ones", cs=(0, nvocab)),
                                 idx_t[:, l0:l0 + cs], start=True, stop=True)
                oh = tp.tile([nvocab, 512], bf16, tag="oh")
                nc.vector.tensor_scalar(out=oh[:, :cs], in0=psb[:, :cs],
                                        scalar1=g("blob_f", "iota", rs=(0, nvocab)),
                                        scalar2=None, op0=ALU.is_equal)
                pse = pp.tile([DIM, 512], f32, tag="ps")
                nc.tensor.matmul(pse[:, :cs], emb_ap, oh[:, :cs], start=True, stop=True)
                nc.scalar.copy(e[:, l0:l0 + cs], pse[:, :cs])
            return e

        pe = embed(idx_p_t, 26, LP, g("blob_m", "embp"), "p")
        de = embed(idx_d_t, 65, LD, g("blob_m", "embd"), "d")

        # ---- conv stacks (bf16 in/out, f32 psum) ----
        def conv(x, Lout, K, w_aps, b_ap, cout, tag):
            y = ap_.tile([cout, Lout], bf16, tag=tag)
            for l0 in range(0, Lout, 512):
                cs = min(512, Lout - l0)
                ps = pp.tile([cout, 512], f32, tag="ps")
                for k in range(K):
                    nc.tensor.matmul(ps[:, :cs], w_aps[k], x[:, l0 + k:l0 + k + cs],
                                     start=(k == 0), stop=(k == K - 1))
                nc.scalar.activation(y[:, l0:l0 + cs], ps[:, :cs], AF.Relu, bias=b_ap)
            return y

        pc1 = conv(pe, LP1, 4, [g("blob_m", f"pw1_{k}") for k in range(4)],
                   g("blob_f", "pb1"), CONV, "pc1")
        pc2 = conv(pc1, LP2, 8, [g("blob_e", f"pw2_{k}") for k in range(8)],
                   g("blob_f", "pb2"), 2 * CONV, "pc2")
        pc = [conv(pc2, LP3, 12,
                   [g("blob_e", f"pw3_{k}", cs=CH[j]) for k in range(12)],
                   g("blob_f", f"pb3{'AB'[j]}"), CH[j][1], f"pc3_{j}")
              for j in range(2)]
        dc1 = conv(de, LD1, 4, [g("blob_m", f"dw1_{k}") for k in range(4)],
                   g("blob_f", "db1"), CONV, "dc1")
        dc2 = conv(dc1, LD2, 6, [g("blob_e", f"dw2_{k}") for k in range(6)],
                   g("blob_f", "db2"), 2 * CONV, "dc2")
        dc = [conv(dc2, LD3, 8,
                   [g("blob_e", f"dw3_{k}", cs=CH[j]) for k in range(8)],
                   g("blob_f", f"db3{'AB'[j]}"), CH[j][1], f"dc3_{j}")
              for j in range(2)]

        # ---- attention projections ----
        # out tiles: X_A [128, L] (chans 0:128) and X_B4 [128, L] (chans 128:160 x4 rep)
        def att_proj(src, L, wkey, bkey, tag, dt_a):
            res = []
            for which in range(2):  # 0 = A, 1 = B4(replicated)
                y = ap_.tile([128, L], dt_a if which == 0 or tag == "D" else bf16,
                             tag=f"{tag}{which}")
                for l0 in range(0, L, 512):
                    cs = min(512, L - l0)
                    ps = pp.tile([128, 512], f32, tag="ps")
                    for j in range(2):
                        w = (g("blob_a", f"{wkey}{'AB'[j]}", cs=(0, 128)) if which == 0
                             else g("blob_a", f"{wkey}r{'AB'[j]}"))
                        nc.tensor.matmul(ps[:, :cs], w, src[j][:, l0:l0 + cs],
                                         start=(j == 0), stop=(j == 1))
                    bias = (g("blob_f", f"{bkey}A") if which == 0
                            else g("blob_f", f"{bkey}r"))
                    nc.scalar.activation(y[:, l0:l0 + cs], ps[:, :cs], AF.Identity,
                                         bias=bias)
                res.append(y)
            return res

        # D tiles f32 (used as per-partition scalars); P tiles bf16 (streamed)
        P_A, P_B4 = att_proj(pc, LP3, "paw", "pab", "P", bf16)
        D_A, D_B4 = att_proj(dc, LD3, "daw", "dab", "D", f32)

        # pack D_B4 [128, 85] -> D_Bp [128, 22]: lane (32g+c), col t = D[128+c, 4t+g]
        D_Bpad = ap_.tile([128, 88], f32, tag="D_Bpad")
        nc.vector.memset(D_Bpad[:], -1e4)
        nc.vector.tensor_copy(D_Bpad[:, 0:85], D_B4[:])
        D_Bp = ap_.tile([128, NB], f32, tag="D_Bp")
        for gi in range(4):
            nc.vector.tensor_copy(D_Bp[gi * 32:(gi + 1) * 32, :],
                                  D_Bpad[gi * 32:(gi + 1) * 32, gi:88:4])

        # ---- R loops ----
        # per iteration: tmp = relu(P + D[:,i]) with fused free-axis accumulation
        # (S column) on ACT or DVE; T += tmp via identity matmul into PSUM.
        # Some DVE pairs are pre-folded (add) so PE streams fewer tiles.
        def r_loop(P_t, D_cols, n_iter, s_tile, psl, psh, id_ap, n_act, n_fold):
            act_set = {int((k + 0.5) * n_iter / n_act) for k in range(n_act)}
            spare = (i for i in range(n_iter) if i not in act_set)
            while len(act_set) < n_act:
                act_set.add(next(spare))
            n_pairs_tot = (n_iter - n_act) // 2
            n_fold = min(n_fold, n_pairs_tot)
            ns = n_iter - n_fold  # total tiles streamed into T psum
            si = 0
            pend = []
            pair_idx = 0

            def stream(t):
                nonlocal si
                nc.tensor.matmul(psl[:], id_ap, t[:, 0:512],
                                 start=(si == 0), stop=(si == ns - 1))
                nc.tensor.matmul(psh[:], id_ap, t[:, 512:LP3],
                                 start=(si == 0), stop=(si == ns - 1))
                si += 1

            folds_done = 0
            for i in range(n_iter):
                tm = tp.tile([128, LP3], bf16, tag="rtmp")
                if i in act_set:
                    nc.scalar.activation(tm[:], P_t[:], AF.Relu,
                                         bias=D_cols[:, i:i + 1],
                                         accum_out=s_tile[:, i:i + 1])
                    stream(tm)
                    continue
                nc.vector.tensor_scalar(out=tm[:], in0=P_t[:],
                                        scalar1=D_cols[:, i:i + 1], scalar2=0.0,
                                        op0=ALU.add, op1=ALU.max,
                                        accum_out=s_tile[:, i:i + 1])
                pend.append(tm)
                if len(pend) == 2:
                    pair_idx += 1
                    do_fold = (folds_done < n_fold and
                               (pair_idx * n_fold) // n_pairs_tot >
                               ((pair_idx - 1) * n_fold) // n_pairs_tot)
                    if do_fold:
                        fo = fp.tile([128, LP3], bf16, tag="rfold")
                        nc.vector.tensor_tensor(out=fo[:], in0=pend[0][:],
                                                in1=pend[1][:], op=ALU.add)
                        stream(fo)
                        folds_done += 1
                    else:
                        stream(pend[0])
                        stream(pend[1])
                    pend = []
            for t in pend:
                stream(t)
            assert si == ns, (si, ns)

        S_A = ap_.tile([128, LD3], f32, tag="S_A")
        TA0 = pT.tile([128, 512], f32, tag="TA0")
        TA1 = pT.tile([128, LP3 - 512], f32, tag="TA1")
        r_loop(P_A, D_A, LD3, S_A, TA0, TA1, g("blob_a", "id128"), NACT_A, NFOLD_A)

        S_B4 = ap_.tile([128, NB], f32, tag="S_B4")
        TB0 = pT.tile([32, 512], f32, tag="TB0")
        TB1 = pT.tile([32, LP3 - 512], f32, tag="TB1")
        r_loop(P_B4, D_Bp, NB, S_B4, TB0, TB1, g("blob_a", "id4"), NACT_B, NFOLD_B)

        # S -> bf16 rhs tiles: S_Ab [128, 85]; unpack S_B4 -> S_Bb [32, 85]
        S_Ab = ap_.tile([128, LD3], bf16, tag="S_Ab")
        nc.vector.tensor_copy(S_Ab[:], S_A[:])
        S_Bb = ap_.tile([32, LD3], bf16, tag="S_Bb")
        for gi in range(4):
            cnt = NB if gi == 0 else NB - 1
            nc.vector.tensor_copy(S_Bb[:, gi:gi + 4 * (cnt - 1) + 1:4],
                                  S_B4[gi * 32:(gi + 1) * 32, 0:cnt])
        # T psum -> bf16 sbuf (split across ACT and DVE)
        T_Ab = ap_.tile([128, LP3], bf16, tag="T_Ab")
        nc.scalar.copy(T_Ab[:, 0:512], TA0[:])
        nc.vector.tensor_copy(T_Ab[:, 512:LP3], TA1[:])
        T_Bb = ap_.tile([32, LP3], bf16, tag="T_Bb")
        nc.scalar.copy(T_Bb[:, 0:512], TB0[:])
        nc.vector.tensor_copy(T_Bb[:, 512:LP3], TB1[:])
        S_ch = [S_Ab, S_Bb]
        T_ch = [T_Ab, T_Bb]

        # ---- attention outputs: sigmoid((sum/n) @ att_w + att_b) ----
        def atte(rhs_ch, L, scale, tag):
            res = []
            for which, (o, w) in enumerate(CH):
                y = ap_.tile([w, L], bf16, tag=f"{tag}{which}")
                for l0 in range(0, L, 512):
                    cs = min(512, L - l0)
                    ps = pp.tile([w, 512], f32, tag="ps")
                    for j in range(2):
                        nc.tensor.matmul(ps[:, :cs],
                                         g("blob_a", f"aw{'AB'[j]}", cs=(o, w)),
                                         rhs_ch[j][:, l0:l0 + cs],
                                         start=(j == 0), stop=(j == 1))
                    nc.scalar.activation(y[:, l0:l0 + cs], ps[:, :cs], AF.Sigmoid,
                                         bias=g("blob_f", f"ab{'AB'[which]}"),
                                         scale=scale)
                res.append(y)
            return res

        ca = atte(S_ch, LD3, 1.0 / LP3, "ca")
        pa = atte(T_ch, LP3, 1.0 / LD3, "pa")

        # ---- gate + global max pool: v = max_l(src * (0.5 + atte)) ----
        vecs = {}
        for (src, att_, L, tag) in [(dc, ca, LD3, "d"), (pc, pa, LP3, "p")]:
            for which, (o, w) in enumerate(CH):
                gt = tp.tile([w, L], bf16, tag=f"g_{tag}{which}")
                nc.vector.tensor_scalar(out=gt[:], in0=att_[which][:], scalar1=0.5,
                                        scalar2=None, op0=ALU.add)
                m = tp.tile([w, L], bf16, tag=f"m_{tag}{which}")
                nc.vector.tensor_tensor(out=m[:], in0=src[which][:], in1=gt[:],
                                        op=ALU.mult)
                v = ap_.tile([w, 1], bf16, tag=f"v_{tag}{which}")
                nc.vector.reduce_max(v[:], m[:], axis=AX.X)
                vecs[f"{tag}{which}"] = v
        # pair layout: [dvecA(128), dvecB(32), pvecA(128), pvecB(32)]
        vlist = [vecs["d0"], vecs["d1"], vecs["p0"], vecs["p1"]]

        # ---- FC head ----
        def lrelu_bias(ps, b_ap, ncols, tag):
            h = ap_.tile([128, ncols], f32, tag=f"h_{tag}")
            nc.vector.tensor_tensor(out=h[:], in0=ps[:, :ncols], in1=b_ap, op=ALU.add)
            t1 = tp.tile([128, ncols], f32, tag="fct")
            nc.vector.tensor_scalar(out=t1[:], in0=h[:], scalar1=0.01, scalar2=None,
                                    op0=ALU.mult)
            h2 = ap_.tile([128, ncols], bf16, tag=f"h2_{tag}")
            nc.vector.tensor_tensor(out=h2[:], in0=h[:], in1=t1[:], op=ALU.max)
            return h2

        ps1 = pp.tile([128, 8], f32, tag="ps")
        for oc in range(8):
            for gi in range(4):
                nc.tensor.matmul(ps1[:, oc:oc + 1],
                                 g("blob_fc", f"fc1w_{gi}", cs=(oc * 128, 128)),
                                 vlist[gi][:], start=(gi == 0), stop=(gi == 3))
        h1 = lrelu_bias(ps1, g("blob_f", "fc1b"), 8, "1")

        ps2 = pp.tile([128, 8], f32, tag="ps")
        for oc in range(8):
            for gi in range(8):
                nc.tensor.matmul(ps2[:, oc:oc + 1],
                                 g("blob_fc", f"fc2w_{gi}", cs=(oc * 128, 128)),
                                 h1[:, gi:gi + 1], start=(gi == 0), stop=(gi == 7))
        h2 = lrelu_bias(ps2, g("blob_f", "fc2b"), 8, "2")

        ps3 = pp.tile([128, 4], f32, tag="ps")
        for oc in range(4):
            for gi in range(8):
                nc.tensor.matmul(ps3[:, oc:oc + 1],
                                 g("blob_fc", f"fc3w_{gi}", cs=(oc * 128, 128)),
                                 h2[:, gi:gi + 1], start=(gi == 0), stop=(gi == 7))
        h3 = lrelu_bias(ps3, g("blob_f", "fc3b"), 4, "3")

        pso = pp.tile([2, 1], f32, tag="ps")
        for gi in range(4):
            nc.tensor.matmul(pso[:], g("blob_fc", f"outw_{gi}"), h3[:, gi:gi + 1],
                             start=(gi == 0), stop=(gi == 3))
        ob = ap_.tile([2, 1], f32, tag="ob")
        nc.scalar.activation(ob[:], pso[:], AF.Identity, bias=g("blob_f", "outb"))
        nc.sync.dma_start(out=out_d[:], in_=ob[:])

    nc.compile()
    return nc


def _prep_inputs(inputs):
    """Host-side layout prep. Returns (shared_params, per_core_fn)."""
    import ml_dtypes
    bf = ml_dtypes.bfloat16
    a = lambda x: np.asarray(x)
    rep4 = lambda x: np.tile(x, (4, 1))

    fills = {}
    fills["ones"] = np.ones((1, 128), np.float32)
    fills["embd"] = a(inputs["drug_emb"])
    fills["embp"] = a(inputs["prot_emb"])
    for pre, w1, w2, w3, k2, k3 in [("d", "dw1", "dw2", "dw3", 6, 8),
                                    ("p", "pw1", "pw2", "pw3", 8, 12)]:
        t1 = a(inputs[w1]).transpose(2, 1, 0)
        t2 = a(inputs[w2]).transpose(2, 1, 0)
        t3 = a(inputs[w3]).transpose(2, 1, 0)
        for k in range(4):
            fills[f"{pre}w1_{k}"] = t1[k]
        for k in range(k2):
            fills[f"{pre}w2_{k}"] = t2[k]
        for k in range(k3):
            fills[f"{pre}w3_{k}"] = t3[k]
    fills["id128"] = np.eye(128, dtype=np.float32)
    fills["id4"] = np.tile(np.eye(32, dtype=np.float32), (4, 1))
    for key, wname in [("daw", "d_att_w"), ("paw", "p_att_w"), ("aw", "att_w")]:
        w = a(inputs[wname])
        fills[f"{key}A"] = w[0:128]
        fills[f"{key}B"] = w[128:160]
    for key, wname in [("dawr", "d_att_w"), ("pawr", "p_att_w")]:
        w = np.tile(a(inputs[wname])[:, 128:160], (1, 4))
        fills[f"{key}A"] = w[0:128]
        fills[f"{key}B"] = w[128:160]
    for i, (o, w) in enumerate(FCCH):
        fills[f"fc1w_{i}"] = a(inputs["fc1_w"])[o:o + w]
    for gi in range(8):
        fills[f"fc2w_{gi}"] = a(inputs["fc2_w"])[gi * 128:(gi + 1) * 128]
        fills[f"fc3w_{gi}"] = a(inputs["fc3_w"])[gi * 128:(gi + 1) * 128]
    for gi in range(4):
        fills[f"outw_{gi}"] = a(inputs["out_w"])[gi * 128:(gi + 1) * 128]
    # f32 blob
    fills["iota"] = np.arange(128, dtype=np.float32).reshape(128, 1)
    for key, bname in [("db1", "db1"), ("db2", "db2"), ("pb1", "pb1"),
                       ("pb2", "pb2")]:
        fills[key] = a(inputs[bname]).reshape(-1, 1)
    for key, bname in [("db3", "db3"), ("pb3", "pb3"), ("dab", "d_att_b"),
                       ("pab", "p_att_b"), ("ab", "att_b")]:
        v = a(inputs[bname]).reshape(-1, 1)
        fills[f"{key}A"] = v[0:128]
        fills[f"{key}B"] = v[128:160]
    fills["dabr"] = rep4(a(inputs["d_att_b"]).reshape(-1, 1)[128:160])
    fills["pabr"] = rep4(a(inputs["p_att_b"]).reshape(-1, 1)[128:160])
    fills["fc1b"] = a(inputs["fc1_b"]).reshape(8, 128).T
    fills["fc2b"] = a(inputs["fc2_b"]).reshape(8, 128).T
    fills["fc3b"] = a(inputs["fc3_b"]).reshape(4, 128).T
    fills["outb"] = a(inputs["out_b"]).reshape(2, 1)

    shared = {}
    for bname, pos, w in BLOBS:
        dt = np.float32 if bname == "blob_f" else bf
        arr = np.zeros((128, w), dt)
        for name, (r, c0, cw) in pos.items():
            arr[0:r, c0:c0 + cw] = fills[name].astype(dt)
        shared[bname] = arr

    drug = a(inputs["drug"]).astype(bf)
    prot = a(inputs["protein"]).astype(bf)

    def per_core(i):
        m = dict(shared)
        m["drug_idx"] = np.ascontiguousarray(drug[i:i + 1])
        m["prot_idx"] = np.ascontiguousarray(prot[i:i + 1])
        return m

    return shared, per_core


def kernel(**inputs):
    from concourse.bass_utils import run_bass_kernel_spmd

    if "nc" not in _CACHE:
        _CACHE["nc"] = _build()
    nc = _CACHE["nc"]
    _, per_core = _prep_inputs(inputs)
    in_maps = [per_core(i) for i in range(B)]
    r = run_bass_kernel_spmd(nc, in_maps, core_ids=list(range(B)))
    out = np.stack([r.results[i]["out"].reshape(2) for i in range(B)])
    return out.astype(np.float32)


# revision 21
# speedup vs baseline: 1.3298x; 1.3298x over previous
"""AttentionDTI forward pass on 8 TRN2 NeuronCores — pure data parallel over batch.

Model (B=8, LD=100, LP=1000, DIM=64, CONV=40, C4=160):
  embed -> 3x conv1d+relu (drug: k=4,6,8 ; protein: k=4,8,12)
  d_att = dc^T @ d_att_w + b ; p_att = pc^T @ p_att_w + b
  R = relu(d_att[:,i,None,:] + p_att[:,None,j,:])      # [B,85,979,160] never materialized
  comp_atte = sigmoid((R.mean(2) @ att_w + att_b)^T)   # via S[c,i] = sum_j relu(...)
  prot_atte = sigmoid((R.mean(1) @ att_w + att_b)^T)   # via T[c,j] = sum_i relu(...)
  gate, global max pool, FC 320->1024->1024->512->2 (leaky relu 0.01)

Sharding: core b handles batch element b. All params replicated. No collectives.

v2 changes vs baseline:
- All parameters packed host-side into a few [128, W] DRAM blobs, loaded with
  single large DMAs (the baseline's 105 small DMAs serialized ~60us on the SP
  sequencer at 565ns each and kept the DMA queue in tiny <2KB packets).
  Input indices go first; FC blob rides the second HWDGE queue (Activation).
- R loop: DVE iterations use tensor_scalar (TensorScalarPtr supports the DVE
  4x perf mode for packed bf16 SBUF operands) instead of scalar_tensor_tensor
  (no perf modes). ACT iterations keep activation+accum. Additionally DVE
  folds some pairs of tmp tiles (tensor_tensor add, 2x mode) so the PE
  identity-matmul T-accumulation streams fewer tiles.
- PE warmup decoupled from DMA (gpsimd memset source) and shortened.
"""

import numpy as np

B, LD, LP, DIM, CONV = 8, 100, 1000, 64, 40
C4 = 160
LD1, LD2, LD3 = 97, 92, 85     # drug conv output lengths (k=4,6,8)
LP1, LP2, LP3 = 997, 990, 979  # protein conv output lengths (k=4,8,12)
NB = 22                        # ceil(85/4) packed iterations for chunk B

# tuning knobs
WARMUP_MM = 10
# R loop: even iterations on ACT (true relu + fused S accum); odd iterations
# on DVE via tensor_tensor_reduce in shifted form max(P,-d) = relu(P+d) - d
# (S and T corrected linearly afterwards). Some shifted pairs are folded
# (tensor_tensor add) on DVE or GpSimd so the PE streams fewer tiles.
RCFG_A = dict(nf_dve=5, nf_pool=0)   # 85 iters -> 43 ACT, 42 TTR, 21 pairs
RCFG_B = dict(nf_dve=1, nf_pool=0)   # 22 iters -> 11 ACT, 11 TTR
R_ODD_OP = "stt"                     # "ttr" (shifted form) or "stt" (fallback)

_CACHE = {}

# ---------------- blob layouts (shared by build and host prep) ----------------
# each entry: (name, rows, cols)
L_M = ([("ones", 1, 128), ("embd", 65, DIM), ("embp", 26, DIM)]
       + [(f"dw1_{k}", DIM, CONV) for k in range(4)]
       + [(f"pw1_{k}", DIM, CONV) for k in range(4)])
L_E = ([(f"pw2_{k}", CONV, 2 * CONV) for k in range(8)]
       + [(f"pw3_{k}", 2 * CONV, C4) for k in range(12)]
       + [(f"dw2_{k}", CONV, 2 * CONV) for k in range(6)]
       + [(f"dw3_{k}", 2 * CONV, C4) for k in range(8)])
L_A = ([("id128", 128, 128), ("id4", 128, 32)]
       + [(f"{w}{c}", 128 if c == "A" else 32, C4)
          for w in ("daw", "paw", "aw") for c in ("A", "B")]
       + [(f"{w}{c}", 128 if c == "A" else 32, 128)
          for w in ("dawr", "pawr") for c in ("A", "B")])
FCCH = [(0, 128), (128, 32), (160, 128), (288, 32)]
L_FC = ([(f"fc1w_{i}", w, 1024) for i, (o, w) in enumerate(FCCH)]
        + [(f"fc2w_{g}", 128, 1024) for g in range(8)]
        + [(f"fc3w_{g}", 128, 512) for g in range(8)]
        + [(f"outw_{g}", 128, 2) for g in range(4)])
L_F = ([("iota", 128, 1), ("db1", CONV, 1), ("db2", 2 * CONV, 1),
        ("db3A", 128, 1), ("db3B", 32, 1), ("pb1", CONV, 1),
        ("pb2", 2 * CONV, 1), ("pb3A", 128, 1), ("pb3B", 32, 1),
        ("dabA", 128, 1), ("dabB", 32, 1), ("pabA", 128, 1), ("pabB", 32, 1),
        ("abA", 128, 1), ("abB", 32, 1), ("dabr", 128, 1), ("pabr", 128, 1),
        ("fc1b", 128, 8), ("fc2b", 128, 8), ("fc3b", 128, 4), ("outb", 2, 1)])


def _layout(items):
    pos, c = {}, 0
    for name, r, w in items:
        pos[name] = (r, c, w)
        c += w
    return pos, c


POS_M, W_M = _layout(L_M)
POS_E, W_E = _layout(L_E)
POS_A, W_A = _layout(L_A)
POS_FC, W_FC = _layout(L_FC)
POS_F, W_F = _layout(L_F)
# DMA issue order: tiny f32 scalars first, then embed/conv weights, then the
# 4.2MB FC blob last so it cannot starve the critical-path transfers.
BLOBS = [("blob_f", POS_F, W_F), ("blob_m", POS_M, W_M), ("blob_e", POS_E, W_E),
         ("blob_a", POS_A, W_A), ("blob_fc", POS_FC, W_FC)]


def _build():
    from contextlib import ExitStack
    import concourse.bass as bass
    import concourse.tile as tile
    from concourse import bacc, mybir

    f32 = mybir.dt.float32
    bf16 = mybir.dt.bfloat16
    AF = mybir.ActivationFunctionType
    ALU = mybir.AluOpType
    AX = mybir.AxisListType

    nc = bacc.Bacc("TRN2", target_bir_lowering=False, debug=False)

    d_idx = nc.declare_dram_parameter("drug_idx", [1, LD], bf16, isOutput=False)
    p_idx = nc.declare_dram_parameter("prot_idx", [1, LP], bf16, isOutput=False)
    blob_d = {}
    for bname, pos, w in BLOBS:
        dt = f32 if bname == "blob_f" else bf16
        blob_d[bname] = nc.declare_dram_parameter(bname, [128, w], dt, isOutput=False)
    out_d = nc.declare_dram_parameter("out", [2, 1], f32, isOutput=True)

    CH = [(0, 128), (128, 32)]  # (offset, width) chunks of the 160 dim

    with tile.TileContext(nc) as tc, ExitStack() as ctx:
        wp = ctx.enter_context(tc.tile_pool(name="w", bufs=1))
        ap_ = ctx.enter_context(tc.tile_pool(name="a", bufs=1))
        tp = ctx.enter_context(tc.tile_pool(name="t", bufs=8))
        fp = ctx.enter_context(tc.tile_pool(name="f", bufs=4))
        pp = ctx.enter_context(tc.tile_pool(name="p", bufs=2, space="PSUM"))
        pT = ctx.enter_context(tc.tile_pool(name="pT", bufs=1, space="PSUM"))

        # ---- blob DMAs; indices first; FC blob on the second HWDGE queue ----
        nc_blob = {}
        idx_d_t = ap_.tile([1, LD], bf16, tag="idx_d")
        nc.sync.dma_start(out=idx_d_t[:], in_=d_idx[:])
        idx_p_t = ap_.tile([1, LP], bf16, tag="idx_p")
        nc.sync.dma_start(out=idx_p_t[:], in_=p_idx[:])
        for bname, pos, w in BLOBS:
            dt = f32 if bname == "blob_f" else bf16
            t = wp.tile([128, w], dt, tag=bname)
            nc.sync.dma_start(out=t[:], in_=blob_d[bname][:])
            nc_blob[bname] = t

        def g(bname, name, rs=None, cs=None):
            """AP for packed tile `name` in blob `bname`, optionally sub-sliced."""
            r, c0, w = {"blob_m": POS_M, "blob_e": POS_E, "blob_a": POS_A,
                        "blob_fc": POS_FC, "blob_f": POS_F}[bname][name]
            r0, r1 = (0, r) if rs is None else (rs[0], rs[1])
            co, cw = (0, w) if cs is None else cs
            return nc_blob[bname][r0:r1, c0 + co:c0 + co + cw]

        # ---- PE warmup while DMAs land: memset source, no DMA dependency ----
        wu = ap_.tile([128, 512], bf16, tag="wu")
        nc.gpsimd.memset(wu[:], 0.0)
        ps_wu = pT.tile([128, 512], mybir.dt.float32, tag="wu")
        for _ in range(WARMUP_MM):
            nc.tensor.matmul(ps_wu[:], wu[:, 0:128], wu[:], start=True, stop=True)

        # ---- one-hot + embedding ----
        def embed(idx_t, nvocab, L, emb_ap, tag):
            e = ap_.tile([DIM, L], bf16, tag=f"e_{tag}")
            for l0 in range(0, L, 512):
                cs = min(512, L - l0)
                psb = pp.tile([nvocab, 512], f32, tag="ps")
                nc.tensor.matmul(psb[:, :cs], g("blob_m", "ones", cs=(0, nvocab)),
                                 idx_t[:, l0:l0 + cs], start=True, stop=True)
                oh = tp.tile([nvocab, 512], bf16, tag="oh")
                nc.vector.tensor_scalar(out=oh[:, :cs], in0=psb[:, :cs],
                                        scalar1=g("blob_f", "iota", rs=(0, nvocab)),
                                        scalar2=None, op0=ALU.is_equal)
                pse = pp.tile([DIM, 512], f32, tag="ps")
                nc.tensor.matmul(pse[:, :cs], emb_ap, oh[:, :cs], start=True, stop=True)
                nc.scalar.copy(e[:, l0:l0 + cs], pse[:, :cs])
            return e

        pe = embed(idx_p_t, 26, LP, g("blob_m", "embp"), "p")
        de = embed(idx_d_t, 65, LD, g("blob_m", "embd"), "d")

        # ---- conv stacks (bf16 in/out, f32 psum) ----
        def conv(x, Lout, K, w_aps, b_ap, cout, tag):
            y = ap_.tile([cout, Lout], bf16, tag=tag)
            for l0 in range(0, Lout, 512):
                cs = min(512, Lout - l0)
                ps = pp.tile([cout, 512], f32, tag="ps")
                for k in range(K):
                    nc.tensor.matmul(ps[:, :cs], w_aps[k], x[:, l0 + k:l0 + k + cs],
                                     start=(k == 0), stop=(k == K - 1))
                nc.scalar.activation(y[:, l0:l0 + cs], ps[:, :cs], AF.Relu, bias=b_ap)
            return y

        pc1 = conv(pe, LP1, 4, [g("blob_m", f"pw1_{k}") for k in range(4)],
                   g("blob_f", "pb1"), CONV, "pc1")
        pc2 = conv(pc1, LP2, 8, [g("blob_e", f"pw2_{k}") for k in range(8)],
                   g("blob_f", "pb2"), 2 * CONV, "pc2")
        pc = [conv(pc2, LP3, 12,
                   [g("blob_e", f"pw3_{k}", cs=CH[j]) for k in range(12)],
                   g("blob_f", f"pb3{'AB'[j]}"), CH[j][1], f"pc3_{j}")
              for j in range(2)]
        dc1 = conv(de, LD1, 4, [g("blob_m", f"dw1_{k}") for k in range(4)],
                   g("blob_f", "db1"), CONV, "dc1")
        dc2 = conv(dc1, LD2, 6, [g("blob_e", f"dw2_{k}") for k in range(6)],
                   g("blob_f", "db2"), 2 * CONV, "dc2")
        dc = [conv(dc2, LD3, 8,
                   [g("blob_e", f"dw3_{k}", cs=CH[j]) for k in range(8)],
                   g("blob_f", f"db3{'AB'[j]}"), CH[j][1], f"dc3_{j}")
              for j in range(2)]

        # ---- attention projections ----
        # out tiles: X_A [128, L] (chans 0:128) and X_B4 [128, L] (chans 128:160 x4 rep)
        def att_proj(src, L, wkey, bkey, tag, dt_a):
            res = []
            for which in range(2):  # 0 = A, 1 = B4(replicated)
                y = ap_.tile([128, L], dt_a if which == 0 or tag == "D" else bf16,
                             tag=f"{tag}{which}")
                for l0 in range(0, L, 512):
                    cs = min(512, L - l0)
                    ps = pp.tile([128, 512], f32, tag="ps")
                    for j in range(2):
                        w = (g("blob_a", f"{wkey}{'AB'[j]}", cs=(0, 128)) if which == 0
                             else g("blob_a", f"{wkey}r{'AB'[j]}"))
                        nc.tensor.matmul(ps[:, :cs], w, src[j][:, l0:l0 + cs],
                                         start=(j == 0), stop=(j == 1))
                    bias = (g("blob_f", f"{bkey}A") if which == 0
                            else g("blob_f", f"{bkey}r"))
                    nc.scalar.activation(y[:, l0:l0 + cs], ps[:, :cs], AF.Identity,
                                         bias=bias)
                res.append(y)
            return res

        # D tiles f32 (used as per-partition scalars); P tiles bf16 (streamed)
        P_A, P_B4 = att_proj(pc, LP3, "paw", "pab", "P", bf16)
        D_A, D_B4 = att_proj(dc, LD3, "daw", "dab", "D", f32)

        # pack D_B4 [128, 85] -> D_Bp [128, 22]: lane (32g+c), col t = D[128+c, 4t+g]
        # pad value -8: P + (-8) < 0 always, and max(P, 8) = 8.0 is bf16-exact so
        # the shifted-form pad contribution cancels exactly against D_masked.
        D_Bpad = ap_.tile([128, 88], f32, tag="D_Bpad")
        nc.vector.memset(D_Bpad[:], -8.0)
        nc.vector.tensor_copy(D_Bpad[:, 0:85], D_B4[:])
        D_Bp = ap_.tile([128, NB], f32, tag="D_Bp")
        for gi in range(4):
            nc.vector.tensor_copy(D_Bp[gi * 32:(gi + 1) * 32, :],
                                  D_Bpad[gi * 32:(gi + 1) * 32, gi:88:4])

        # ---- R loops ----
        # Even iterations (ACT): tmp = relu(P + d_i) with fused S-column accum.
        # Odd iterations (DVE tensor_tensor_reduce): tmp' = max(P, -d_i)
        #   = relu(P + d_i) - d_i, with fused accum S'[:,i] = sum_j tmp'.
        # T psum accumulates tmp/tmp' via identity matmuls; afterwards
        #   T_true[c,j] = T_psum[c,j] + sum_{i odd} d[c,i]   (per-lane bias)
        #   S_eff[c,i] = S_raw[c,i]/979 + (d[c,i] if i odd else 0)
        # both corrections ride existing copy ops (bias add / masked D add).
        # Some odd pairs are folded (tensor_tensor add) on DVE or GpSimd so
        # the PE streams fewer tiles.
        def r_loop(P_t, D_cols, negD, n_iter, s_tile, psl, psh, id_ap, cfg):
            n_ttr = n_iter // 2
            n_pairs = n_ttr // 2
            exs = []  # alternate executors so neither engine gets a burst
            p, dv = cfg["nf_pool"], cfg["nf_dve"]
            while (p or dv) and len(exs) < n_pairs:
                if p:
                    exs.append("pool")
                    p -= 1
                if dv and len(exs) < n_pairs:
                    exs.append("dve")
                    dv -= 1
            folds = [None] * n_pairs
            for k, ex in enumerate(exs):  # spread folded pairs evenly
                folds[int((k + 0.5) * n_pairs / len(exs))] = ex
            n_f = sum(1 for f in folds if f)
            ns = n_iter - n_f
            si = 0
            pend = []
            pair_idx = 0

            def stream(t):
                nonlocal si
                nc.tensor.matmul(psl[:], id_ap, t[:, 0:512],
                                 start=(si == 0), stop=(si == ns - 1))
                nc.tensor.matmul(psh[:], id_ap, t[:, 512:LP3],
                                 start=(si == 0), stop=(si == ns - 1))
                si += 1

            for i in range(n_iter):
                tm = tp.tile([128, LP3], bf16, tag="rtmp")
                if i % 2 == 0:
                    nc.scalar.activation(tm[:], P_t[:], AF.Relu,
                                         bias=D_cols[:, i:i + 1],
                                         accum_out=s_tile[:, i:i + 1])
                    stream(tm)
                    continue
                if R_ODD_OP == "ttr":
                    nc.vector.tensor_tensor_reduce(
                        out=tm[:], in0=P_t[:],
                        in1=negD[:, i:i + 1].broadcast_to((128, LP3)),
                        scale=1.0, scalar=0.0, op0=ALU.max, op1=ALU.add,
                        accum_out=s_tile[:, i:i + 1])
                else:
                    # same shifted form via scalar_tensor_tensor:
                    # max(P, -d) + 0, fused row-sum accum
                    nc.vector.scalar_tensor_tensor(
                        out=tm[:], in0=P_t[:], scalar=negD[:, i:i + 1],
                        in1=zeros_t[:], op0=ALU.max, op1=ALU.add,
                        accum_out=s_tile[:, i:i + 1])
                pend.append(tm)
                if len(pend) == 2:
                    ex = folds[pair_idx] if pair_idx < n_pairs else None
                    pair_idx += 1
                    if ex:
                        fo = fp.tile([128, LP3], bf16, tag="rfold")
                        eng = nc.vector if ex == "dve" else nc.gpsimd
                        eng.tensor_tensor(out=fo[:], in0=pend[0][:],
                                          in1=pend[1][:], op=ALU.add)
                        stream(fo)
                    else:
                        stream(pend[0])
                        stream(pend[1])
                    pend = []
            for t in pend:
                stream(t)
            assert si == ns, (si, ns)

        # negated D columns (TTR in1) and odd-masked D (corrections)
        def neg_mask(D_cols, L, tag):
            nD = ap_.tile([128, L], f32, tag=f"nD_{tag}")
            nc.vector.tensor_scalar(out=nD[:], in0=D_cols[:], scalar1=-1.0,
                                    scalar2=None, op0=ALU.mult)
            Dm = ap_.tile([128, L], f32, tag=f"Dm_{tag}")
            nc.vector.memset(Dm[:], 0.0)
            nc.vector.tensor_copy(Dm[:, 1::2], D_cols[:, 1::2])
            return nD, Dm

        negD_A, Dm_A = neg_mask(D_A, LD3, "A")
        negD_B, Dm_B = neg_mask(D_Bp, NB, "B")
        if R_ODD_OP == "stt":
            zeros_t = ap_.tile([128, LP3], bf16, tag="zeros")
            nc.vector.memset(zeros_t[:], 0.0)

        S_A = ap_.tile([128, LD3], f32, tag="S_A")
        TA0 = pT.tile([128, 512], f32, tag="TA0")
        TA1 = pT.tile([128, LP3 - 512], f32, tag="TA1")
        r_loop(P_A, D_A, negD_A, LD3, S_A, TA0, TA1, g("blob_a", "id128"), RCFG_A)

        S_B4 = ap_.tile([128, NB], f32, tag="S_B4")
        TB0 = pT.tile([32, 512], f32, tag="TB0")
        TB1 = pT.tile([32, LP3 - 512], f32, tag="TB1")
        r_loop(P_B4, D_Bp, negD_B, NB, S_B4, TB0, TB1, g("blob_a", "id4"), RCFG_B)

        # T bias corrections: dsum[c] = sum_{i odd} d[c,i]
        dsA = ap_.tile([128, 1], f32, tag="dsA")
        nc.vector.reduce_sum(dsA[:], Dm_A[:], axis=AX.X)
        dsB4f = ap_.tile([128, 1], f32, tag="dsB4f")
        nc.vector.reduce_sum(dsB4f[:], Dm_B[:], axis=AX.X)
        dsB4 = ap_.tile([128, 1], bf16, tag="dsB4")
        nc.vector.tensor_copy(dsB4[:], dsB4f[:])
        psds = pp.tile([32, 1], f32, tag="ps")
        nc.tensor.matmul(psds[:], g("blob_a", "id4"), dsB4[:], start=True, stop=True)
        dsB = ap_.tile([32, 1], f32, tag="dsB")
        nc.vector.tensor_copy(dsB[:], psds[:])

        # S_eff = S_raw/979 + masked D  -> bf16 rhs tiles (atte ca uses scale 1)
        S_Ab = ap_.tile([128, LD3], bf16, tag="S_Ab")
        nc.vector.scalar_tensor_tensor(out=S_Ab[:], in0=S_A[:], scalar=1.0 / LP3,
                                       op0=ALU.mult, in1=Dm_A[:], op1=ALU.add)
        S_B4e = ap_.tile([128, NB], bf16, tag="S_B4e")
        nc.vector.scalar_tensor_tensor(out=S_B4e[:], in0=S_B4[:], scalar=1.0 / LP3,
                                       op0=ALU.mult, in1=Dm_B[:], op1=ALU.add)
        S_Bb = ap_.tile([32, LD3], bf16, tag="S_Bb")
        for gi in range(4):
            cnt = NB if gi == 0 else NB - 1
            nc.vector.tensor_copy(S_Bb[:, gi:gi + 4 * (cnt - 1) + 1:4],
                                  S_B4e[gi * 32:(gi + 1) * 32, 0:cnt])
        # T psum -> bf16 sbuf with the dsum bias (split across ACT and DVE)
        T_Ab = ap_.tile([128, LP3], bf16, tag="T_Ab")
        nc.scalar.activation(T_Ab[:, 0:512], TA0[:], AF.Identity, bias=dsA[:])
        nc.vector.tensor_scalar(out=T_Ab[:, 512:LP3], in0=TA1[:], scalar1=dsA[:],
                                scalar2=None, op0=ALU.add)
        T_Bb = ap_.tile([32, LP3], bf16, tag="T_Bb")
        nc.scalar.activation(T_Bb[:, 0:512], TB0[:], AF.Identity, bias=dsB[:])
        nc.vector.tensor_scalar(out=T_Bb[:, 512:LP3], in0=TB1[:], scalar1=dsB[:],
                                scalar2=None, op0=ALU.add)
        S_ch = [S_Ab, S_Bb]
        T_ch = [T_Ab, T_Bb]

        # ---- attention outputs: sigmoid((sum/n) @ att_w + att_b) ----
        def atte(rhs_ch, L, scale, tag):
            res = []
            for which, (o, w) in enumerate(CH):
                y = ap_.tile([w, L], bf16, tag=f"{tag}{which}")
                for l0 in range(0, L, 512):
                    cs = min(512, L - l0)
                    ps = pp.tile([w, 512], f32, tag="ps")
                    for j in range(2):
                        nc.tensor.matmul(ps[:, :cs],
                                         g("blob_a", f"aw{'AB'[j]}", cs=(o, w)),
                                         rhs_ch[j][:, l0:l0 + cs],
                                         start=(j == 0), stop=(j == 1))
                    nc.scalar.activation(y[:, l0:l0 + cs], ps[:, :cs], AF.Sigmoid,
                                         bias=g("blob_f", f"ab{'AB'[which]}"),
                                         scale=scale)
                res.append(y)
            return res

        ca = atte(S_ch, LD3, 1.0, "ca")  # S_eff already divided by LP3
        pa = atte(T_ch, LP3, 1.0 / LD3, "pa")

        # ---- gate + global max pool: v = max_l(src * (0.5 + atte)) ----
        vecs = {}
        for (src, att_, L, tag) in [(dc, ca, LD3, "d"), (pc, pa, LP3, "p")]:
            for which, (o, w) in enumerate(CH):
                gt = tp.tile([w, L], bf16, tag=f"g_{tag}{which}")
                nc.vector.tensor_scalar(out=gt[:], in0=att_[which][:], scalar1=0.5,
                                        scalar2=None, op0=ALU.add)
                m = tp.tile([w, L], bf16, tag=f"m_{tag}{which}")
                nc.vector.tensor_tensor(out=m[:], in0=src[which][:], in1=gt[:],
                                        op=ALU.mult)
                v = ap_.tile([w, 1], bf16, tag=f"v_{tag}{which}")
                nc.vector.reduce_max(v[:], m[:], axis=AX.X)
                vecs[f"{tag}{which}"] = v
        # pair layout: [dvecA(128), dvecB(32), pvecA(128), pvecB(32)]
        vlist = [vecs["d0"], vecs["d1"], vecs["p0"], vecs["p1"]]

        # ---- FC head ----
        def lrelu_bias(ps, b_ap, ncols, tag):
            h = ap_.tile([128, ncols], f32, tag=f"h_{tag}")
            nc.vector.tensor_tensor(out=h[:], in0=ps[:, :ncols], in1=b_ap, op=ALU.add)
            t1 = tp.tile([128, ncols], f32, tag="fct")
            nc.vector.tensor_scalar(out=t1[:], in0=h[:], scalar1=0.01, scalar2=None,
                                    op0=ALU.mult)
            h2 = ap_.tile([128, ncols], bf16, tag=f"h2_{tag}")
            nc.vector.tensor_tensor(out=h2[:], in0=h[:], in1=t1[:], op=ALU.max)
            return h2

        ps1 = pp.tile([128, 8], f32, tag="ps")
        for oc in range(8):
            for gi in range(4):
                nc.tensor.matmul(ps1[:, oc:oc + 1],
                                 g("blob_fc", f"fc1w_{gi}", cs=(oc * 128, 128)),
                                 vlist[gi][:], start=(gi == 0), stop=(gi == 3))
        h1 = lrelu_bias(ps1, g("blob_f", "fc1b"), 8, "1")

        ps2 = pp.tile([128, 8], f32, tag="ps")
        for oc in range(8):
            for gi in range(8):
                nc.tensor.matmul(ps2[:, oc:oc + 1],
                                 g("blob_fc", f"fc2w_{gi}", cs=(oc * 128, 128)),
                                 h1[:, gi:gi + 1], start=(gi == 0), stop=(gi == 7))
        h2 = lrelu_bias(ps2, g("blob_f", "fc2b"), 8, "2")

        ps3 = pp.tile([128, 4], f32, tag="ps")
        for oc in range(4):
            for gi in range(8):
                nc.tensor.matmul(ps3[:, oc:oc + 1],
                                 g("blob_fc", f"fc3w_{gi}", cs=(oc * 128, 128)),
                                 h2[:, gi:gi + 1], start=(gi == 0), stop=(gi == 7))
        h3 = lrelu_bias(ps3, g("blob_f", "fc3b"), 4, "3")

        pso = pp.tile([2, 1], f32, tag="ps")
        for gi in range(4):
            nc.tensor.matmul(pso[:], g("blob_fc", f"outw_{gi}"), h3[:, gi:gi + 1],
                             start=(gi == 0), stop=(gi == 3))
        ob = ap_.tile([2, 1], f32, tag="ob")
        nc.scalar.activation(ob[:], pso[:], AF.Identity, bias=g("blob_f", "outb"))
        nc.sync.dma_start(out=out_d[:], in_=ob[:])

    nc.compile()
    return nc


def _prep_inputs(inputs):
    """Host-side layout prep. Returns (shared_params, per_core_fn)."""
    import ml_dtypes
    bf = ml_dtypes.bfloat16
    a = lambda x: np.asarray(x)
    rep4 = lambda x: np.tile(x, (4, 1))

    fills = {}
    fills["ones"] = np.ones((1, 128), np.float32)
    fills["embd"] = a(inputs["drug_emb"])
    fills["embp"] = a(inputs["prot_emb"])
    for pre, w1, w2, w3, k2, k3 in [("d", "dw1", "dw2", "dw3", 6, 8),
                                    ("p", "pw1", "pw2", "pw3", 8, 12)]:
        t1 = a(inputs[w1]).transpose(2, 1, 0)
        t2 = a(inputs[w2]).transpose(2, 1, 0)
        t3 = a(inputs[w3]).transpose(2, 1, 0)
        for k in range(4):
            fills[f"{pre}w1_{k}"] = t1[k]
        for k in range(k2):
            fills[f"{pre}w2_{k}"] = t2[k]
        for k in range(k3):
            fills[f"{pre}w3_{k}"] = t3[k]
    fills["id128"] = np.eye(128, dtype=np.float32)
    fills["id4"] = np.tile(np.eye(32, dtype=np.float32), (4, 1))
    for key, wname in [("daw", "d_att_w"), ("paw", "p_att_w"), ("aw", "att_w")]:
        w = a(inputs[wname])
        fills[f"{key}A"] = w[0:128]
        fills[f"{key}B"] = w[128:160]
    for key, wname in [("dawr", "d_att_w"), ("pawr", "p_att_w")]:
        w = np.tile(a(inputs[wname])[:, 128:160], (1, 4))
        fills[f"{key}A"] = w[0:128]
        fills[f"{key}B"] = w[128:160]
    for i, (o, w) in enumerate(FCCH):
        fills[f"fc1w_{i}"] = a(inputs["fc1_w"])[o:o + w]
    for gi in range(8):
        fills[f"fc2w_{gi}"] = a(inputs["fc2_w"])[gi * 128:(gi + 1) * 128]
        fills[f"fc3w_{gi}"] = a(inputs["fc3_w"])[gi * 128:(gi + 1) * 128]
    for gi in range(4):
        fills[f"outw_{gi}"] = a(inputs["out_w"])[gi * 128:(gi + 1) * 128]
    # f32 blob
    fills["iota"] = np.arange(128, dtype=np.float32).reshape(128, 1)
    for key, bname in [("db1", "db1"), ("db2", "db2"), ("pb1", "pb1"),
                       ("pb2", "pb2")]:
        fills[key] = a(inputs[bname]).reshape(-1, 1)
    for key, bname in [("db3", "db3"), ("pb3", "pb3"), ("dab", "d_att_b"),
                       ("pab", "p_att_b"), ("ab", "att_b")]:
        v = a(inputs[bname]).reshape(-1, 1)
        fills[f"{key}A"] = v[0:128]
        fills[f"{key}B"] = v[128:160]
    fills["dabr"] = rep4(a(inputs["d_att_b"]).reshape(-1, 1)[128:160])
    fills["pabr"] = rep4(a(inputs["p_att_b"]).reshape(-1, 1)[128:160])
    fills["fc1b"] = a(inputs["fc1_b"]).reshape(8, 128).T
    fills["fc2b"] = a(inputs["fc2_b"]).reshape(8, 128).T
    fills["fc3b"] = a(inputs["fc3_b"]).reshape(4, 128).T
    fills["outb"] = a(inputs["out_b"]).reshape(2, 1)

    shared = {}
    for bname, pos, w in BLOBS:
        dt = np.float32 if bname == "blob_f" else bf
        arr = np.zeros((128, w), dt)
        for name, (r, c0, cw) in pos.items():
            arr[0:r, c0:c0 + cw] = fills[name].astype(dt)
        shared[bname] = arr

    drug = a(inputs["drug"]).astype(bf)
    prot = a(inputs["protein"]).astype(bf)

    def per_core(i):
        m = dict(shared)
        m["drug_idx"] = np.ascontiguousarray(drug[i:i + 1])
        m["prot_idx"] = np.ascontiguousarray(prot[i:i + 1])
        return m

    return shared, per_core


def kernel(**inputs):
    from concourse.bass_utils import run_bass_kernel_spmd

    if "nc" not in _CACHE:
        _CACHE["nc"] = _build()
    nc = _CACHE["nc"]
    _, per_core = _prep_inputs(inputs)
    in_maps = [per_core(i) for i in range(B)]
    r = run_bass_kernel_spmd(nc, in_maps, core_ids=list(range(B)))
    out = np.stack([r.results[i]["out"].reshape(2) for i in range(B)])
    return out.astype(np.float32)


# revision 49
# speedup vs baseline: 1.4007x; 1.0533x over previous
"""AttentionDTI forward pass on 8 TRN2 NeuronCores — pure data parallel over batch.

Model (B=8, LD=100, LP=1000, DIM=64, CONV=40, C4=160):
  embed -> 3x conv1d+relu (drug: k=4,6,8 ; protein: k=4,8,12)
  d_att = dc^T @ d_att_w + b ; p_att = pc^T @ p_att_w + b
  R = relu(d_att[:,i,None,:] + p_att[:,None,j,:])      # [B,85,979,160] never materialized
  comp_atte = sigmoid((R.mean(2) @ att_w + att_b)^T)   # via S[c,i] = sum_j relu(...)
  prot_atte = sigmoid((R.mean(1) @ att_w + att_b)^T)   # via T[c,j] = sum_i relu(...)
  gate, global max pool, FC 320->1024->1024->512->2 (leaky relu 0.01)

Sharding: core b handles batch element b. All params replicated. No collectives.

v2 changes vs baseline:
- All parameters packed host-side into a few [128, W] DRAM blobs, loaded with
  single large DMAs (the baseline's 105 small DMAs serialized ~60us on the SP
  sequencer at 565ns each and kept the DMA queue in tiny <2KB packets).
  Input indices go first; FC blob rides the second HWDGE queue (Activation).
- R loop: DVE iterations use tensor_scalar (TensorScalarPtr supports the DVE
  4x perf mode for packed bf16 SBUF operands) instead of scalar_tensor_tensor
  (no perf modes). ACT iterations keep activation+accum. Additionally DVE
  folds some pairs of tmp tiles (tensor_tensor add, 2x mode) so the PE
  identity-matmul T-accumulation streams fewer tiles.
- PE warmup decoupled from DMA (gpsimd memset source) and shortened.
"""

import numpy as np

B, LD, LP, DIM, CONV = 8, 100, 1000, 64, 40
C4 = 160
LD1, LD2, LD3 = 97, 92, 85     # drug conv output lengths (k=4,6,8)
LP1, LP2, LP3 = 997, 990, 979  # protein conv output lengths (k=4,8,12)
NB = 22                        # ceil(85/4) packed iterations for chunk B

# tuning knobs
WARMUP_MM = 10
# R loop: even iterations on ACT (true relu + fused S accum); odd iterations
# on DVE via tensor_tensor_reduce in shifted form max(P,-d) = relu(P+d) - d
# (S and T corrected linearly afterwards). Some shifted pairs are folded
# (tensor_tensor add) on DVE or GpSimd so the PE streams fewer tiles.
RCFG_A = dict(nf_dve=0, nf_pool=0)   # 85 iters -> 43 ACT, 42 shifted-DVE
RCFG_B = dict(nf_dve=0, nf_pool=0)   # 22 iters -> 11 ACT, 11 shifted-DVE
R_ODD_OP = "stt"                     # "ttr" (shifted form) or "stt" (fallback)
QS = 16.0                            # fp8 scale for FC weights and activations

_CACHE = {}

# ---------------- blob layouts (shared by build and host prep) ----------------
# each entry: (name, rows, cols). Conv taps are pre-stacked in pairs along the
# contract dim (conv via x2 tiles that hold [x ; x shifted left 1]).
L_M = ([("ones", 1, 128), ("embd", 65, DIM), ("embp", 26, DIM)]
       + [(f"dw1s_{k}", 2 * DIM, CONV) for k in range(2)]
       + [(f"pw1s_{k}", 2 * DIM, CONV) for k in range(2)])
# conv2 stacked weights are [104, 80]: tap 2k rows 0:40, zeros 40:64 (the x2
# tile's unwritten rows), tap 2k+1 rows 64:104 (partition-base-64 aligned).
L_E = ([(f"pw2s_{k}", 104, 2 * CONV) for k in range(4)]
       + [(f"pw3_{k}", 2 * CONV, C4) for k in range(12)]
       + [(f"dw2s_{k}", 104, 2 * CONV) for k in range(3)]
       + [(f"dw3_{k}", 2 * CONV, C4) for k in range(8)])
L_A = ([("id128", 128, 128), ("id4", 128, 32),
        ("dmaskA", 128, LD3), ("dmaskB", 128, NB)]
       + [(f"outw_{gi}", 128, 2) for gi in range(4)]
       + [(f"{w}{c}", 128 if c == "A" else 32, C4)
          for w in ("daw", "paw", "aw") for c in ("A", "B")]
       + [(f"{w}{c}", 128 if c == "A" else 32, 128)
          for w in ("dawr", "pawr") for c in ("A", "B")])
FCCH = [(0, 128), (128, 32), (160, 128), (288, 32)]
L_Q = ([(f"fc1w_{i}", w, 1024) for i, (o, w) in enumerate(FCCH)]
       + [(f"fc2w_{gi}", 128, 1024) for gi in range(8)]
       + [(f"fc3w_{gi}", 128, 512) for gi in range(8)])
L_F = ([("half", 128, 1), ("iota", 128, 1), ("db1", CONV, 1), ("db2", 2 * CONV, 1),
        ("db3A", 128, 1), ("db3B", 32, 1), ("pb1", CONV, 1),
        ("pb2", 2 * CONV, 1), ("pb3A", 128, 1), ("pb3B", 32, 1),
        ("dabA", 128, 1), ("dabB", 32, 1), ("pabA", 128, 1), ("pabB", 32, 1),
        ("abA", 128, 1), ("abB", 32, 1), ("dabr", 128, 1), ("pabr", 128, 1),
        ("fc1b", 128, 8), ("fc2b", 128, 8), ("fc3b", 128, 4), ("outb", 2, 1)])


def _layout(items):
    pos, c = {}, 0
    for name, r, w in items:
        pos[name] = (r, c, w)
        c += w
    return pos, c


POS_M, W_M = _layout(L_M)
POS_E, W_E = _layout(L_E)
POS_A, W_A = _layout(L_A)
POS_Q, W_Q = _layout(L_Q)
POS_F, W_F = _layout(L_F)
# DMA issue order: tiny f32 scalars first, then embed/conv weights, then the
# FC blob last so it cannot starve the critical-path transfers.
BLOBS = [("blob_f", POS_F, W_F), ("blob_m", POS_M, W_M), ("blob_e", POS_E, W_E),
         ("blob_a", POS_A, W_A), ("blob_q", POS_Q, W_Q)]


def _build():
    from contextlib import ExitStack
    import concourse.bass as bass
    import concourse.tile as tile
    from concourse import bacc, mybir

    f32 = mybir.dt.float32
    bf16 = mybir.dt.bfloat16
    AF = mybir.ActivationFunctionType
    ALU = mybir.AluOpType
    AX = mybir.AxisListType

    nc = bacc.Bacc("TRN2", target_bir_lowering=False, debug=False)

    bdt = {"blob_f": f32}

    d_idx = nc.declare_dram_parameter("drug_idx", [1, LD], bf16, isOutput=False)
    p_idx = nc.declare_dram_parameter("prot_idx", [1, LP], bf16, isOutput=False)
    blob_d = {}
    for bname, pos, w in BLOBS:
        dt = bdt.get(bname, bf16)
        blob_d[bname] = nc.declare_dram_parameter(bname, [128, w], dt, isOutput=False)
    out_d = nc.declare_dram_parameter("out", [2, 1], f32, isOutput=True)

    CH = [(0, 128), (128, 32)]  # (offset, width) chunks of the 160 dim

    with tile.TileContext(nc) as tc, ExitStack() as ctx:
        wp = ctx.enter_context(tc.tile_pool(name="w", bufs=1))
        ap_ = ctx.enter_context(tc.tile_pool(name="a", bufs=1))
        tp = ctx.enter_context(tc.tile_pool(name="t", bufs=8))
        fp = ctx.enter_context(tc.tile_pool(name="f", bufs=4))
        pp = ctx.enter_context(tc.tile_pool(name="p", bufs=2, space="PSUM"))
        pT = ctx.enter_context(tc.tile_pool(name="pT", bufs=1, space="PSUM"))

        # ---- blob DMAs; indices first; FC blob on the second HWDGE queue ----
        nc_blob = {}
        idx_d_t = ap_.tile([1, LD], bf16, tag="idx_d")
        nc.sync.dma_start(out=idx_d_t[:], in_=d_idx[:])
        idx_p_t = ap_.tile([1, LP], bf16, tag="idx_p")
        nc.sync.dma_start(out=idx_p_t[:], in_=p_idx[:])
        for bname, pos, w in BLOBS:
            dt = bdt.get(bname, bf16)
            t = wp.tile([128, w], dt, tag=bname)
            nc.sync.dma_start(out=t[:], in_=blob_d[bname][:])
            nc_blob[bname] = t

        def g(bname, name, rs=None, cs=None):
            """AP for packed tile `name` in blob `bname`, optionally sub-sliced."""
            r, c0, w = {"blob_m": POS_M, "blob_e": POS_E, "blob_a": POS_A,
                        "blob_q": POS_Q, "blob_f": POS_F}[bname][name]
            r0, r1 = (0, r) if rs is None else (rs[0], rs[1])
            co, cw = (0, w) if cs is None else cs
            return nc_blob[bname][r0:r1, c0 + co:c0 + co + cw]

        # ---- PE warmup while DMAs land: memset source, no DMA dependency ----
        wu = ap_.tile([128, 512], bf16, tag="wu")
        nc.gpsimd.memset(wu[:], 0.0)
        ps_wu = pT.tile([128, 512], mybir.dt.float32, tag="wu")
        for _ in range(WARMUP_MM):
            nc.tensor.matmul(ps_wu[:], wu[:, 0:128], wu[:], start=True, stop=True)
        # preload the sigmoid activation table now so the one-time
        # ACT_TABLE_LOAD (~1.3us) is not serialized into the tail
        wu_s = ap_.tile([1, 1], bf16, tag="wu_s")
        nc.scalar.activation(wu_s[:], wu[0:1, 0:1], AF.Sigmoid)

        # ---- one-hot + embedding (written into the top rows of an x2 tile) ----
        def embed(idx_t, nvocab, L, emb_ap, out_t):
            for l0 in range(0, L, 512):
                cs = min(512, L - l0)
                psb = pp.tile([nvocab, 512], f32, tag="ps")
                nc.tensor.matmul(psb[:, :cs], g("blob_m", "ones", cs=(0, nvocab)),
                                 idx_t[:, l0:l0 + cs], start=True, stop=True)
                oh = tp.tile([nvocab, 512], bf16, tag="oh")
                nc.vector.tensor_scalar(out=oh[:, :cs], in0=psb[:, :cs],
                                        scalar1=g("blob_f", "iota", rs=(0, nvocab)),
                                        scalar2=None, op0=ALU.is_equal)
                pse = pp.tile([DIM, 512], f32, tag="ps")
                nc.tensor.matmul(pse[:, :cs], emb_ap, oh[:, :cs], start=True, stop=True)
                nc.scalar.copy(out_t[0:DIM, l0:l0 + cs], pse[:, :cs])

        def shift2(x2, rows, L):
            """x2[64:64+rows, c] = x2[0:rows, c+1] — builds the stacked-tap input.
            The shifted block sits at partition 64 (engine writes need a
            32-aligned partition base)."""
            nc.vector.tensor_copy(x2[64:64 + rows, 0:L - 1], x2[0:rows, 1:L])

        pe2 = ap_.tile([128, LP], bf16, tag="pe2")
        embed(idx_p_t, 26, LP, g("blob_m", "embp"), pe2)
        shift2(pe2, DIM, LP)
        de2 = ap_.tile([128, LD], bf16, tag="de2")
        embed(idx_d_t, 65, LD, g("blob_m", "embd"), de2)
        shift2(de2, DIM, LD)

        # ---- conv stacks (bf16 in/out, f32 psum); step=2 for stacked taps ----
        def conv(x, Lout, K, w_aps, b_ap, cout, tag, step=1, out=None):
            y = out if out is not None else ap_.tile([cout, Lout], bf16, tag=tag)
            crows = w_aps[0].partition_size()
            for l0 in range(0, Lout, 512):
                cs = min(512, Lout - l0)
                ps = pp.tile([cout, 512], f32, tag="ps")
                for k in range(K):
                    nc.tensor.matmul(ps[:, :cs], w_aps[k],
                                     x[0:crows, l0 + step * k:l0 + step * k + cs],
                                     start=(k == 0), stop=(k == K - 1))
                nc.scalar.activation(y[0:cout, l0:l0 + cs], ps[:, :cs],
                                     AF.Relu, bias=b_ap)
            return y

        pc1x2 = ap_.tile([128, LP1], bf16, tag="pc1x2")
        nc.vector.memset(pc1x2[32:64, :], 0.0)
        conv(pe2, LP1, 2, [g("blob_m", f"pw1s_{k}") for k in range(2)],
             g("blob_f", "pb1"), CONV, "pc1", step=2, out=pc1x2)
        shift2(pc1x2, CONV, LP1)
        pc2 = conv(pc1x2, LP2, 4, [g("blob_e", f"pw2s_{k}") for k in range(4)],
                   g("blob_f", "pb2"), 2 * CONV, "pc2", step=2)
        pc = [conv(pc2, LP3, 12,
                   [g("blob_e", f"pw3_{k}", cs=CH[j]) for k in range(12)],
                   g("blob_f", f"pb3{'AB'[j]}"), CH[j][1], f"pc3_{j}")
              for j in range(2)]
        dc1x2 = ap_.tile([128, LD1], bf16, tag="dc1x2")
        nc.vector.memset(dc1x2[32:64, :], 0.0)
        conv(de2, LD1, 2, [g("blob_m", f"dw1s_{k}") for k in range(2)],
             g("blob_f", "db1"), CONV, "dc1", step=2, out=dc1x2)
        shift2(dc1x2, CONV, LD1)
        dc2 = conv(dc1x2, LD2, 3, [g("blob_e", f"dw2s_{k}") for k in range(3)],
                   g("blob_f", "db2"), 2 * CONV, "dc2", step=2)
        dc = [conv(dc2, LD3, 8,
                   [g("blob_e", f"dw3_{k}", cs=CH[j]) for k in range(8)],
                   g("blob_f", f"db3{'AB'[j]}"), CH[j][1], f"dc3_{j}")
              for j in range(2)]

        # ---- attention projections ----
        # out tiles: X_A [128, L] (chans 0:128) and X_B4 [128, L] (chans 128:160 x4 rep)
        def att_proj(src, L, wkey, bkey, tag, dt_a):
            res = []
            for which in range(2):  # 0 = A, 1 = B4(replicated)
                y = ap_.tile([128, L], dt_a if which == 0 or tag == "D" else bf16,
                             tag=f"{tag}{which}")
                for l0 in range(0, L, 512):
                    cs = min(512, L - l0)
                    ps = pp.tile([128, 512], f32, tag="ps")
                    for j in range(2):
                        w = (g("blob_a", f"{wkey}{'AB'[j]}", cs=(0, 128)) if which == 0
                             else g("blob_a", f"{wkey}r{'AB'[j]}"))
                        nc.tensor.matmul(ps[:, :cs], w, src[j][:, l0:l0 + cs],
                                         start=(j == 0), stop=(j == 1))
                    bias = (g("blob_f", f"{bkey}A") if which == 0
                            else g("blob_f", f"{bkey}r"))
                    nc.scalar.activation(y[:, l0:l0 + cs], ps[:, :cs], AF.Identity,
                                         bias=bias)
                res.append(y)
            return res

        # D tiles f32 (used as per-partition scalars); P tiles bf16 (streamed)
        P_A, P_B4 = att_proj(pc, LP3, "paw", "pab", "P", bf16)
        D_A, D_B4 = att_proj(dc, LD3, "daw", "dab", "D", f32)

        # pack D_B4 [128, 85] -> D_Bp [128, 22]: lane (32g+c), col t = D[128+c, 4t+g]
        # pad value -8: P + (-8) < 0 always, and max(P, 8) = 8.0 is bf16-exact so
        # the shifted-form pad contribution cancels exactly against D_masked.
        D_Bpad = ap_.tile([128, 88], f32, tag="D_Bpad")
        nc.vector.memset(D_Bpad[:], -8.0)
        nc.vector.tensor_copy(D_Bpad[:, 0:85], D_B4[:])
        D_Bp = ap_.tile([128, NB], f32, tag="D_Bp")
        for gi in range(4):
            nc.vector.tensor_copy(D_Bp[gi * 32:(gi + 1) * 32, :],
                                  D_Bpad[gi * 32:(gi + 1) * 32, gi:88:4])

        # ---- R loops ----
        # Even iterations (ACT): tmp = relu(P + d_i) with fused S-column accum.
        # Odd iterations (DVE tensor_tensor_reduce): tmp' = max(P, -d_i)
        #   = relu(P + d_i) - d_i, with fused accum S'[:,i] = sum_j tmp'.
        # T psum accumulates tmp/tmp' via identity matmuls; afterwards
        #   T_true[c,j] = T_psum[c,j] + sum_{i odd} d[c,i]   (per-lane bias)
        #   S_eff[c,i] = S_raw[c,i]/979 + (d[c,i] if i odd else 0)
        # both corrections ride existing copy ops (bias add / masked D add).
        # Some odd pairs are folded (tensor_tensor add) on DVE or GpSimd so
        # the PE streams fewer tiles.
        def r_loop(P_t, D_cols, negD, n_iter, s_tile, psl, psh, id_ap, cfg):
            n_ttr = n_iter // 2
            n_pairs = n_ttr // 2
            exs = []  # alternate executors so neither engine gets a burst
            p, dv = cfg["nf_pool"], cfg["nf_dve"]
            while (p or dv) and len(exs) < n_pairs:
                if p:
                    exs.append("pool")
                    p -= 1
                if dv and len(exs) < n_pairs:
                    exs.append("dve")
                    dv -= 1
            folds = [None] * n_pairs
            for k, ex in enumerate(exs):  # spread folded pairs evenly
                folds[int((k + 0.5) * n_pairs / len(exs))] = ex
            n_f = sum(1 for f in folds if f)
            ns = n_iter - n_f
            si = 0
            pend = []
            pair_idx = 0

            def stream(t):
                nonlocal si
                nc.tensor.matmul(psl[:], id_ap, t[:, 0:512],
                                 start=(si == 0), stop=(si == ns - 1))
                nc.tensor.matmul(psh[:], id_ap, t[:, 512:LP3],
                                 start=(si == 0), stop=(si == ns - 1))
                si += 1

            for i in range(n_iter):
                tm = tp.tile([128, LP3], bf16, tag="rtmp")
                if i % 2 == 0:
                    nc.scalar.activation(tm[:], P_t[:], AF.Relu,
                                         bias=D_cols[:, i:i + 1],
                                         accum_out=s_tile[:, i:i + 1])
                    stream(tm)
                    continue
                if R_ODD_OP == "ttr":
                    nc.vector.tensor_tensor_reduce(
                        out=tm[:], in0=P_t[:],
                        in1=negD[:, i:i + 1].broadcast_to((128, LP3)),
                        scale=1.0, scalar=0.0, op0=ALU.max, op1=ALU.add,
                        accum_out=s_tile[:, i:i + 1])
                else:
                    # same shifted form via scalar_tensor_tensor:
                    # max(P, -d) + 0, fused row-sum accum
                    nc.vector.scalar_tensor_tensor(
                        out=tm[:], in0=P_t[:], scalar=negD[:, i:i + 1],
                        in1=zeros_t[:], op0=ALU.max, op1=ALU.add,
                        accum_out=s_tile[:, i:i + 1])
                pend.append(tm)
                if len(pend) == 2:
                    ex = folds[pair_idx] if pair_idx < n_pairs else None
                    pair_idx += 1
                    if ex:
                        fo = fp.tile([128, LP3], bf16, tag="rfold")
                        eng = nc.vector if ex == "dve" else nc.gpsimd
                        eng.tensor_tensor(out=fo[:], in0=pend[0][:],
                                          in1=pend[1][:], op=ALU.add)
                        stream(fo)
                    else:
                        stream(pend[0])
                        stream(pend[1])
                    pend = []
            for t in pend:
                stream(t)
            assert si == ns, (si, ns)

        # negated D columns (shifted-form scalar) and odd-masked D (corrections)
        def neg_mask(D_cols, L, tag, mask_ap):
            nD = ap_.tile([128, L], f32, tag=f"nD_{tag}")
            nc.vector.tensor_scalar(out=nD[:], in0=D_cols[:], scalar1=-1.0,
                                    scalar2=None, op0=ALU.mult)
            Dm = ap_.tile([128, L], f32, tag=f"Dm_{tag}")
            nc.vector.tensor_tensor(out=Dm[:], in0=D_cols[:], in1=mask_ap,
                                    op=ALU.mult)
            return nD, Dm

        negD_A, Dm_A = neg_mask(D_A, LD3, "A", g("blob_a", "dmaskA"))
        negD_B, Dm_B = neg_mask(D_Bp, NB, "B", g("blob_a", "dmaskB"))
        if R_ODD_OP == "stt":
            zeros_t = ap_.tile([128, LP3], bf16, tag="zeros")
            nc.vector.memset(zeros_t[:], 0.0)

        S_A = ap_.tile([128, LD3], f32, tag="S_A")
        TA0 = pT.tile([128, 512], f32, tag="TA0")
        TA1 = pT.tile([128, LP3 - 512], f32, tag="TA1")
        r_loop(P_A, D_A, negD_A, LD3, S_A, TA0, TA1, g("blob_a", "id128"), RCFG_A)

        S_B4 = ap_.tile([128, NB], f32, tag="S_B4")
        TB0 = pT.tile([32, 512], f32, tag="TB0")
        TB1 = pT.tile([32, LP3 - 512], f32, tag="TB1")
        r_loop(P_B4, D_Bp, negD_B, NB, S_B4, TB0, TB1, g("blob_a", "id4"), RCFG_B)

        # T bias corrections: dsum[c] = sum_{i odd} d[c,i]
        dsA = ap_.tile([128, 1], f32, tag="dsA")
        nc.vector.reduce_sum(dsA[:], Dm_A[:], axis=AX.X)
        dsB4f = ap_.tile([128, 1], f32, tag="dsB4f")
        nc.vector.reduce_sum(dsB4f[:], Dm_B[:], axis=AX.X)
        dsB4 = ap_.tile([128, 1], bf16, tag="dsB4")
        nc.vector.tensor_copy(dsB4[:], dsB4f[:])
        psds = pp.tile([32, 1], f32, tag="ps")
        nc.tensor.matmul(psds[:], g("blob_a", "id4"), dsB4[:], start=True, stop=True)
        dsB = ap_.tile([32, 1], f32, tag="dsB")
        nc.vector.tensor_copy(dsB[:], psds[:])

        # S_eff = S_raw/979 + masked D  -> bf16 rhs tiles (atte ca uses scale 1)
        S_Ab = ap_.tile([128, LD3], bf16, tag="S_Ab")
        nc.vector.scalar_tensor_tensor(out=S_Ab[:], in0=S_A[:], scalar=1.0 / LP3,
                                       op0=ALU.mult, in1=Dm_A[:], op1=ALU.add)
        S_B4e = ap_.tile([128, NB], bf16, tag="S_B4e")
        nc.vector.scalar_tensor_tensor(out=S_B4e[:], in0=S_B4[:], scalar=1.0 / LP3,
                                       op0=ALU.mult, in1=Dm_B[:], op1=ALU.add)
        S_Bb = ap_.tile([32, LD3], bf16, tag="S_Bb")
        for gi in range(4):
            cnt = NB if gi == 0 else NB - 1
            nc.vector.tensor_copy(S_Bb[:, gi:gi + 4 * (cnt - 1) + 1:4],
                                  S_B4e[gi * 32:(gi + 1) * 32, 0:cnt])
        # T psum -> bf16 sbuf with the dsum bias (split across ACT and DVE)
        T_Ab = ap_.tile([128, LP3], bf16, tag="T_Ab")
        nc.scalar.activation(T_Ab[:, 0:512], TA0[:], AF.Identity, bias=dsA[:])
        nc.vector.tensor_scalar(out=T_Ab[:, 512:LP3], in0=TA1[:], scalar1=dsA[:],
                                scalar2=None, op0=ALU.add)
        T_Bb = ap_.tile([32, LP3], bf16, tag="T_Bb")
        nc.scalar.activation(T_Bb[:, 0:512], TB0[:], AF.Identity, bias=dsB[:])
        nc.vector.tensor_scalar(out=T_Bb[:, 512:LP3], in0=TB1[:], scalar1=dsB[:],
                                scalar2=None, op0=ALU.add)
        S_ch = [S_Ab, S_Bb]
        T_ch = [T_Ab, T_Bb]

        # ---- attention outputs: sigmoid((sum/n) @ att_w + att_b) ----
        def atte(rhs_ch, L, scale, tag):
            res = []
            for which, (o, w) in enumerate(CH):
                y = ap_.tile([w, L], bf16, tag=f"{tag}{which}")
                for l0 in range(0, L, 512):
                    cs = min(512, L - l0)
                    ps = pp.tile([w, 512], f32, tag="ps")
                    for j in range(2):
                        nc.tensor.matmul(ps[:, :cs],
                                         g("blob_a", f"aw{'AB'[j]}", cs=(o, w)),
                                         rhs_ch[j][:, l0:l0 + cs],
                                         start=(j == 0), stop=(j == 1))
                    nc.scalar.activation(y[:, l0:l0 + cs], ps[:, :cs], AF.Sigmoid,
                                         bias=g("blob_f", f"ab{'AB'[which]}"),
                                         scale=scale)
                res.append(y)
            return res

        ca = atte(S_ch, LD3, 1.0, "ca")  # S_eff already divided by LP3
        pa = atte(T_ch, LP3, 1.0 / LD3, "pa")

        # ---- gate + global max pool: v = max_l(src * (0.5 + atte)) ----
        # gt = atte + 0.5 on ACT (idle here); m and the max-reduce on DVE
        vecs = {}
        for (src, att_, L, tag) in [(dc, ca, LD3, "d"), (pc, pa, LP3, "p")]:
            for which, (o, w) in enumerate(CH):
                gt = tp.tile([w, L], bf16, tag=f"g_{tag}{which}")
                nc.scalar.activation(gt[:], att_[which][:], AF.Identity,
                                     bias=g("blob_f", "half", rs=(0, w)))
                m = tp.tile([w, L], bf16, tag=f"m_{tag}{which}")
                nc.vector.tensor_tensor(out=m[:], in0=src[which][:], in1=gt[:],
                                        op=ALU.mult)
                v = ap_.tile([w, 1], bf16, tag=f"v_{tag}{which}")
                nc.vector.reduce_max(v[:], m[:], axis=AX.X)
                vecs[f"{tag}{which}"] = v
        # pair layout: [dvecA(128), dvecB(32), pvecA(128), pvecB(32)]
        vlist = [vecs["d0"], vecs["d1"], vecs["p0"], vecs["p1"]]

        # ---- FC head (bf16 weight-stationary) ----
        def lrelu_bias(ps, b_ap, ncols, tag):
            h = ap_.tile([128, ncols], f32, tag=f"h_{tag}")
            nc.vector.tensor_tensor(out=h[:], in0=ps[:, :ncols], in1=b_ap,
                                    op=ALU.add)
            t1 = tp.tile([128, ncols], f32, tag="fct")
            nc.vector.tensor_scalar(out=t1[:], in0=h[:], scalar1=0.01,
                                    scalar2=None, op0=ALU.mult)
            h2 = ap_.tile([128, ncols], bf16, tag=f"h2_{tag}")
            nc.vector.tensor_tensor(out=h2[:], in0=h[:], in1=t1[:], op=ALU.max)
            return h2

        ps1 = pp.tile([128, 8], f32, tag="ps")
        for oc in range(8):
            for gi in range(4):
                nc.tensor.matmul(ps1[:, oc:oc + 1],
                                 g("blob_q", f"fc1w_{gi}", cs=(oc * 128, 128)),
                                 vlist[gi][:], start=(gi == 0), stop=(gi == 3))
        h1 = lrelu_bias(ps1, g("blob_f", "fc1b"), 8, "1")

        ps2 = pp.tile([128, 8], f32, tag="ps")
        for oc in range(8):
            for gi in range(8):
                nc.tensor.matmul(ps2[:, oc:oc + 1],
                                 g("blob_q", f"fc2w_{gi}", cs=(oc * 128, 128)),
                                 h1[:, gi:gi + 1], start=(gi == 0), stop=(gi == 7))
        h2 = lrelu_bias(ps2, g("blob_f", "fc2b"), 8, "2")

        ps3 = pp.tile([128, 4], f32, tag="ps")
        for oc in range(4):
            for gi in range(8):
                nc.tensor.matmul(ps3[:, oc:oc + 1],
                                 g("blob_q", f"fc3w_{gi}", cs=(oc * 128, 128)),
                                 h2[:, gi:gi + 1], start=(gi == 0), stop=(gi == 7))
        h3 = lrelu_bias(ps3, g("blob_f", "fc3b"), 4, "3")

        pso = pp.tile([2, 1], f32, tag="ps")
        for gi in range(4):
            nc.tensor.matmul(pso[:], g("blob_a", f"outw_{gi}"), h3[:, gi:gi + 1],
                             start=(gi == 0), stop=(gi == 3))
        ob = ap_.tile([2, 1], f32, tag="ob")
        nc.scalar.activation(ob[:], pso[:], AF.Identity, bias=g("blob_f", "outb"))
        nc.sync.dma_start(out=out_d[:], in_=ob[:])

    nc.compile()
    return nc


def _prep_inputs(inputs):
    """Host-side layout prep. Returns (shared_params, per_core_fn)."""
    import ml_dtypes
    bf = ml_dtypes.bfloat16
    a = lambda x: np.asarray(x)
    rep4 = lambda x: np.tile(x, (4, 1))

    fills = {}
    fills["ones"] = np.ones((1, 128), np.float32)
    fills["embd"] = a(inputs["drug_emb"])
    fills["embp"] = a(inputs["prot_emb"])
    for pre, w1, w2, w3, k2s, k3 in [("d", "dw1", "dw2", "dw3", 3, 8),
                                     ("p", "pw1", "pw2", "pw3", 4, 12)]:
        t1 = a(inputs[w1]).transpose(2, 1, 0)
        t2 = a(inputs[w2]).transpose(2, 1, 0)
        t3 = a(inputs[w3]).transpose(2, 1, 0)
        z24 = np.zeros((24, 2 * CONV), np.float32)
        for k in range(2):
            fills[f"{pre}w1s_{k}"] = np.vstack([t1[2 * k], t1[2 * k + 1]])
        for k in range(k2s):
            fills[f"{pre}w2s_{k}"] = np.vstack([t2[2 * k], z24, t2[2 * k + 1]])
        for k in range(k3):
            fills[f"{pre}w3_{k}"] = t3[k]
    fills["id128"] = np.eye(128, dtype=np.float32)
    fills["id4"] = np.tile(np.eye(32, dtype=np.float32), (4, 1))
    mA = np.zeros((128, LD3), np.float32)
    mA[:, 1::2] = 1.0
    mB = np.zeros((128, NB), np.float32)
    mB[:, 1::2] = 1.0
    fills["dmaskA"], fills["dmaskB"] = mA, mB
    for key, wname in [("daw", "d_att_w"), ("paw", "p_att_w"), ("aw", "att_w")]:
        w = a(inputs[wname])
        fills[f"{key}A"] = w[0:128]
        fills[f"{key}B"] = w[128:160]
    for key, wname in [("dawr", "d_att_w"), ("pawr", "p_att_w")]:
        w = np.tile(a(inputs[wname])[:, 128:160], (1, 4))
        fills[f"{key}A"] = w[0:128]
        fills[f"{key}B"] = w[128:160]
    for i, (o, w) in enumerate(FCCH):
        fills[f"fc1w_{i}"] = a(inputs["fc1_w"])[o:o + w]
    for gi in range(8):
        fills[f"fc2w_{gi}"] = a(inputs["fc2_w"])[gi * 128:(gi + 1) * 128]
        fills[f"fc3w_{gi}"] = a(inputs["fc3_w"])[gi * 128:(gi + 1) * 128]
    for gi in range(4):
        fills[f"outw_{gi}"] = a(inputs["out_w"])[gi * 128:(gi + 1) * 128]
    # f32 blob
    fills["half"] = np.full((128, 1), 0.5, np.float32)
    fills["iota"] = np.arange(128, dtype=np.float32).reshape(128, 1)
    for key, bname in [("db1", "db1"), ("db2", "db2"), ("pb1", "pb1"),
                       ("pb2", "pb2")]:
        fills[key] = a(inputs[bname]).reshape(-1, 1)
    for key, bname in [("db3", "db3"), ("pb3", "pb3"), ("dab", "d_att_b"),
                       ("pab", "p_att_b"), ("ab", "att_b")]:
        v = a(inputs[bname]).reshape(-1, 1)
        fills[f"{key}A"] = v[0:128]
        fills[f"{key}B"] = v[128:160]
    fills["dabr"] = rep4(a(inputs["d_att_b"]).reshape(-1, 1)[128:160])
    fills["pabr"] = rep4(a(inputs["p_att_b"]).reshape(-1, 1)[128:160])
    fills["fc1b"] = a(inputs["fc1_b"]).reshape(8, 128).T
    fills["fc2b"] = a(inputs["fc2_b"]).reshape(8, 128).T
    fills["fc3b"] = a(inputs["fc3_b"]).reshape(4, 128).T
    fills["outb"] = a(inputs["out_b"]).reshape(2, 1)

    bdt = {"blob_f": np.float32}
    shared = {}
    for bname, pos, w in BLOBS:
        dt = bdt.get(bname, bf)
        arr = np.zeros((128, w), dt)
        for name, (r, c0, cw) in pos.items():
            arr[0:r, c0:c0 + cw] = fills[name].astype(dt)
        shared[bname] = arr

    drug = a(inputs["drug"]).astype(bf)
    prot = a(inputs["protein"]).astype(bf)

    def per_core(i):
        m = dict(shared)
        m["drug_idx"] = np.ascontiguousarray(drug[i:i + 1])
        m["prot_idx"] = np.ascontiguousarray(prot[i:i + 1])
        return m

    return shared, per_core


def kernel(**inputs):
    from concourse.bass_utils import run_bass_kernel_spmd

    if "nc" not in _CACHE:
        _CACHE["nc"] = _build()
    nc = _CACHE["nc"]
    _, per_core = _prep_inputs(inputs)
    in_maps = [per_core(i) for i in range(B)]
    r = run_bass_kernel_spmd(nc, in_maps, core_ids=list(range(B)))
    out = np.stack([r.results[i]["out"].reshape(2) for i in range(B)])
    return out.astype(np.float32)


# revision 51
# speedup vs baseline: 1.4028x; 1.0015x over previous
"""AttentionDTI forward pass on 8 TRN2 NeuronCores — pure data parallel over batch.

Model (B=8, LD=100, LP=1000, DIM=64, CONV=40, C4=160):
  embed -> 3x conv1d+relu (drug: k=4,6,8 ; protein: k=4,8,12)
  d_att = dc^T @ d_att_w + b ; p_att = pc^T @ p_att_w + b
  R = relu(d_att[:,i,None,:] + p_att[:,None,j,:])      # [B,85,979,160] never materialized
  comp_atte = sigmoid((R.mean(2) @ att_w + att_b)^T)   # via S[c,i] = sum_j relu(...)
  prot_atte = sigmoid((R.mean(1) @ att_w + att_b)^T)   # via T[c,j] = sum_i relu(...)
  gate, global max pool, FC 320->1024->1024->512->2 (leaky relu 0.01)

Sharding: core b handles batch element b. All params replicated. No collectives.

v2 changes vs baseline:
- All parameters packed host-side into a few [128, W] DRAM blobs, loaded with
  single large DMAs (the baseline's 105 small DMAs serialized ~60us on the SP
  sequencer at 565ns each and kept the DMA queue in tiny <2KB packets).
  Input indices go first; FC blob rides the second HWDGE queue (Activation).
- R loop: DVE iterations use tensor_scalar (TensorScalarPtr supports the DVE
  4x perf mode for packed bf16 SBUF operands) instead of scalar_tensor_tensor
  (no perf modes). ACT iterations keep activation+accum. Additionally DVE
  folds some pairs of tmp tiles (tensor_tensor add, 2x mode) so the PE
  identity-matmul T-accumulation streams fewer tiles.
- PE warmup decoupled from DMA (gpsimd memset source) and shortened.
"""

import numpy as np

B, LD, LP, DIM, CONV = 8, 100, 1000, 64, 40
C4 = 160
LD1, LD2, LD3 = 97, 92, 85     # drug conv output lengths (k=4,6,8)
LP1, LP2, LP3 = 997, 990, 979  # protein conv output lengths (k=4,8,12)
NB = 22                        # ceil(85/4) packed iterations for chunk B

# tuning knobs
WARMUP_MM = 10
# R loop: even iterations on ACT (true relu + fused S accum); odd iterations
# on DVE via tensor_tensor_reduce in shifted form max(P,-d) = relu(P+d) - d
# (S and T corrected linearly afterwards). Some shifted pairs are folded
# (tensor_tensor add) on DVE or GpSimd so the PE streams fewer tiles.
RCFG_A = dict(nf_dve=0, nf_pool=0)   # 85 iters -> 43 ACT, 42 shifted-DVE
RCFG_B = dict(nf_dve=0, nf_pool=0)   # 22 iters -> 11 ACT, 11 shifted-DVE
R_ODD_OP = "stt"                     # "ttr" (shifted form) or "stt" (fallback)
QS = 16.0                            # fp8 scale for FC weights and activations

_CACHE = {}

# ---------------- blob layouts (shared by build and host prep) ----------------
# each entry: (name, rows, cols). Conv taps are pre-stacked in pairs along the
# contract dim (conv via x2 tiles that hold [x ; x shifted left 1]).
L_M = ([("ones", 1, 128), ("embd", 65, DIM), ("embp", 26, DIM)]
       + [(f"dw1s_{k}", 2 * DIM, CONV) for k in range(2)]
       + [(f"pw1s_{k}", 2 * DIM, CONV) for k in range(2)])
# conv2 stacked weights are [104, 80]: tap 2k rows 0:40, zeros 40:64 (the x2
# tile's unwritten rows), tap 2k+1 rows 64:104 (partition-base-64 aligned).
L_E = ([(f"pw2s_{k}", 104, 2 * CONV) for k in range(4)]
       + [(f"pw3_{k}", 2 * CONV, C4) for k in range(12)]
       + [(f"dw2s_{k}", 104, 2 * CONV) for k in range(3)]
       + [(f"dw3_{k}", 2 * CONV, C4) for k in range(8)])
L_A = ([("id128", 128, 128), ("id4", 128, 32),
        ("dmaskA", 128, LD3), ("dmaskB", 128, NB)]
       + [(f"outw_{gi}", 128, 2) for gi in range(4)]
       + [(f"{w}{c}", 128 if c == "A" else 32, C4)
          for w in ("daw", "paw", "aw") for c in ("A", "B")]
       + [(f"{w}{c}", 128 if c == "A" else 32, 128)
          for w in ("dawr", "pawr") for c in ("A", "B")])
FCCH = [(0, 128), (128, 32), (160, 128), (288, 32)]
L_Q = ([(f"fc1w_{i}", w, 1024) for i, (o, w) in enumerate(FCCH)]
       + [(f"fc2w_{gi}", 128, 1024) for gi in range(8)]
       + [(f"fc3w_{gi}", 128, 512) for gi in range(8)])
L_F = ([("half", 128, 1), ("iota", 128, 1), ("db1", CONV, 1), ("db2", 2 * CONV, 1),
        ("db3A", 128, 1), ("db3B", 32, 1), ("pb1", CONV, 1),
        ("pb2", 2 * CONV, 1), ("pb3A", 128, 1), ("pb3B", 32, 1),
        ("dabA", 128, 1), ("dabB", 32, 1), ("pabA", 128, 1), ("pabB", 32, 1),
        ("abA", 128, 1), ("abB", 32, 1), ("dabr", 128, 1), ("pabr", 128, 1),
        ("fc1b", 128, 8), ("fc2b", 128, 8), ("fc3b", 128, 4), ("outb", 2, 1)])


def _layout(items):
    pos, c = {}, 0
    for name, r, w in items:
        pos[name] = (r, c, w)
        c += w
    return pos, c


POS_M, W_M = _layout(L_M)
POS_E, W_E = _layout(L_E)
POS_A, W_A = _layout(L_A)
POS_Q, W_Q = _layout(L_Q)
POS_F, W_F = _layout(L_F)
# DMA issue order: tiny f32 scalars first, then embed/conv weights, then the
# FC blob last so it cannot starve the critical-path transfers.
BLOBS = [("blob_f", POS_F, W_F), ("blob_m", POS_M, W_M), ("blob_e", POS_E, W_E),
         ("blob_a", POS_A, W_A), ("blob_q", POS_Q, W_Q)]


def _build():
    from contextlib import ExitStack
    import concourse.bass as bass
    import concourse.tile as tile
    from concourse import bacc, mybir

    f32 = mybir.dt.float32
    bf16 = mybir.dt.bfloat16
    AF = mybir.ActivationFunctionType
    ALU = mybir.AluOpType
    AX = mybir.AxisListType

    nc = bacc.Bacc("TRN2", target_bir_lowering=False, debug=False)

    bdt = {"blob_f": f32}

    d_idx = nc.declare_dram_parameter("drug_idx", [1, LD], bf16, isOutput=False)
    p_idx = nc.declare_dram_parameter("prot_idx", [1, LP], bf16, isOutput=False)
    blob_d = {}
    for bname, pos, w in BLOBS:
        dt = bdt.get(bname, bf16)
        blob_d[bname] = nc.declare_dram_parameter(bname, [128, w], dt, isOutput=False)
    out_d = nc.declare_dram_parameter("out", [2, 1], f32, isOutput=True)

    CH = [(0, 128), (128, 32)]  # (offset, width) chunks of the 160 dim

    with tile.TileContext(nc) as tc, ExitStack() as ctx:
        wp = ctx.enter_context(tc.tile_pool(name="w", bufs=1))
        ap_ = ctx.enter_context(tc.tile_pool(name="a", bufs=1))
        tp = ctx.enter_context(tc.tile_pool(name="t", bufs=8))
        fp = ctx.enter_context(tc.tile_pool(name="f", bufs=4))
        pp = ctx.enter_context(tc.tile_pool(name="p", bufs=2, space="PSUM"))
        pT = ctx.enter_context(tc.tile_pool(name="pT", bufs=1, space="PSUM"))

        # ---- blob DMAs; indices first; FC blob on the second HWDGE queue ----
        nc_blob = {}
        idx_d_t = ap_.tile([1, LD], bf16, tag="idx_d")
        nc.sync.dma_start(out=idx_d_t[:], in_=d_idx[:])
        idx_p_t = ap_.tile([1, LP], bf16, tag="idx_p")
        nc.sync.dma_start(out=idx_p_t[:], in_=p_idx[:])
        for bname, pos, w in BLOBS:
            dt = bdt.get(bname, bf16)
            t = wp.tile([128, w], dt, tag=bname)
            if bname != "blob_q":
                # blob_q (FC weights, 4.2MB) is DMA'd later, once the R loop
                # has started: heavy DMA during the conv phase trips the HAM
                # power throttle and halves the PE clock.
                nc.sync.dma_start(out=t[:], in_=blob_d[bname][:])
            nc_blob[bname] = t

        def g(bname, name, rs=None, cs=None):
            """AP for packed tile `name` in blob `bname`, optionally sub-sliced."""
            r, c0, w = {"blob_m": POS_M, "blob_e": POS_E, "blob_a": POS_A,
                        "blob_q": POS_Q, "blob_f": POS_F}[bname][name]
            r0, r1 = (0, r) if rs is None else (rs[0], rs[1])
            co, cw = (0, w) if cs is None else cs
            return nc_blob[bname][r0:r1, c0 + co:c0 + co + cw]

        # ---- PE warmup while DMAs land: memset source, no DMA dependency ----
        wu = ap_.tile([128, 512], bf16, tag="wu")
        nc.gpsimd.memset(wu[:], 0.0)
        ps_wu = pT.tile([128, 512], mybir.dt.float32, tag="wu")
        for _ in range(WARMUP_MM):
            nc.tensor.matmul(ps_wu[:], wu[:, 0:128], wu[:], start=True, stop=True)
        # preload the sigmoid activation table now so the one-time
        # ACT_TABLE_LOAD (~1.3us) is not serialized into the tail
        wu_s = ap_.tile([1, 1], bf16, tag="wu_s")
        nc.scalar.activation(wu_s[:], wu[0:1, 0:1], AF.Sigmoid)

        # ---- one-hot + embedding (written into the top rows of an x2 tile) ----
        def embed(idx_t, nvocab, L, emb_ap, out_t):
            for l0 in range(0, L, 512):
                cs = min(512, L - l0)
                psb = pp.tile([nvocab, 512], f32, tag="ps")
                nc.tensor.matmul(psb[:, :cs], g("blob_m", "ones", cs=(0, nvocab)),
                                 idx_t[:, l0:l0 + cs], start=True, stop=True)
                oh = tp.tile([nvocab, 512], bf16, tag="oh")
                nc.vector.tensor_scalar(out=oh[:, :cs], in0=psb[:, :cs],
                                        scalar1=g("blob_f", "iota", rs=(0, nvocab)),
                                        scalar2=None, op0=ALU.is_equal)
                pse = pp.tile([DIM, 512], f32, tag="ps")
                nc.tensor.matmul(pse[:, :cs], emb_ap, oh[:, :cs], start=True, stop=True)
                nc.scalar.copy(out_t[0:DIM, l0:l0 + cs], pse[:, :cs])

        def shift2(x2, rows, L):
            """x2[64:64+rows, c] = x2[0:rows, c+1] — builds the stacked-tap input.
            The shifted block sits at partition 64 (engine writes need a
            32-aligned partition base)."""
            nc.vector.tensor_copy(x2[64:64 + rows, 0:L - 1], x2[0:rows, 1:L])

        pe2 = ap_.tile([128, LP], bf16, tag="pe2")
        embed(idx_p_t, 26, LP, g("blob_m", "embp"), pe2)
        shift2(pe2, DIM, LP)
        de2 = ap_.tile([128, LD], bf16, tag="de2")
        embed(idx_d_t, 65, LD, g("blob_m", "embd"), de2)
        shift2(de2, DIM, LD)

        # ---- conv stacks (bf16 in/out, f32 psum); step=2 for stacked taps ----
        def conv(x, Lout, K, w_aps, b_ap, cout, tag, step=1, out=None):
            y = out if out is not None else ap_.tile([cout, Lout], bf16, tag=tag)
            crows = w_aps[0].partition_size()
            for l0 in range(0, Lout, 512):
                cs = min(512, Lout - l0)
                ps = pp.tile([cout, 512], f32, tag="ps")
                for k in range(K):
                    nc.tensor.matmul(ps[:, :cs], w_aps[k],
                                     x[0:crows, l0 + step * k:l0 + step * k + cs],
                                     start=(k == 0), stop=(k == K - 1))
                nc.scalar.activation(y[0:cout, l0:l0 + cs], ps[:, :cs],
                                     AF.Relu, bias=b_ap)
            return y

        pc1x2 = ap_.tile([128, LP1], bf16, tag="pc1x2")
        nc.vector.memset(pc1x2[32:64, :], 0.0)
        conv(pe2, LP1, 2, [g("blob_m", f"pw1s_{k}") for k in range(2)],
             g("blob_f", "pb1"), CONV, "pc1", step=2, out=pc1x2)
        shift2(pc1x2, CONV, LP1)
        pc2 = conv(pc1x2, LP2, 4, [g("blob_e", f"pw2s_{k}") for k in range(4)],
                   g("blob_f", "pb2"), 2 * CONV, "pc2", step=2)
        pc = [conv(pc2, LP3, 12,
                   [g("blob_e", f"pw3_{k}", cs=CH[j]) for k in range(12)],
                   g("blob_f", f"pb3{'AB'[j]}"), CH[j][1], f"pc3_{j}")
              for j in range(2)]
        dc1x2 = ap_.tile([128, LD1], bf16, tag="dc1x2")
        nc.vector.memset(dc1x2[32:64, :], 0.0)
        conv(de2, LD1, 2, [g("blob_m", f"dw1s_{k}") for k in range(2)],
             g("blob_f", "db1"), CONV, "dc1", step=2, out=dc1x2)
        shift2(dc1x2, CONV, LD1)
        dc2 = conv(dc1x2, LD2, 3, [g("blob_e", f"dw2s_{k}") for k in range(3)],
                   g("blob_f", "db2"), 2 * CONV, "dc2", step=2)
        dc = [conv(dc2, LD3, 8,
                   [g("blob_e", f"dw3_{k}", cs=CH[j]) for k in range(8)],
                   g("blob_f", f"db3{'AB'[j]}"), CH[j][1], f"dc3_{j}")
              for j in range(2)]

        # ---- attention projections ----
        # out tiles: X_A [128, L] (chans 0:128) and X_B4 [128, L] (chans 128:160 x4 rep)
        def att_proj(src, L, wkey, bkey, tag, dt_a):
            res = []
            for which in range(2):  # 0 = A, 1 = B4(replicated)
                y = ap_.tile([128, L], dt_a if which == 0 or tag == "D" else bf16,
                             tag=f"{tag}{which}")
                for l0 in range(0, L, 512):
                    cs = min(512, L - l0)
                    ps = pp.tile([128, 512], f32, tag="ps")
                    for j in range(2):
                        w = (g("blob_a", f"{wkey}{'AB'[j]}", cs=(0, 128)) if which == 0
                             else g("blob_a", f"{wkey}r{'AB'[j]}"))
                        nc.tensor.matmul(ps[:, :cs], w, src[j][:, l0:l0 + cs],
                                         start=(j == 0), stop=(j == 1))
                    bias = (g("blob_f", f"{bkey}A") if which == 0
                            else g("blob_f", f"{bkey}r"))
                    nc.scalar.activation(y[:, l0:l0 + cs], ps[:, :cs], AF.Identity,
                                         bias=bias)
                res.append(y)
            return res

        # D tiles f32 (used as per-partition scalars); P tiles bf16 (streamed)
        P_A, P_B4 = att_proj(pc, LP3, "paw", "pab", "P", bf16)
        D_A, D_B4 = att_proj(dc, LD3, "daw", "dab", "D", f32)

        # pack D_B4 [128, 85] -> D_Bp [128, 22]: lane (32g+c), col t = D[128+c, 4t+g]
        # pad value -8: P + (-8) < 0 always, and max(P, 8) = 8.0 is bf16-exact so
        # the shifted-form pad contribution cancels exactly against D_masked.
        D_Bpad = ap_.tile([128, 88], f32, tag="D_Bpad")
        nc.vector.memset(D_Bpad[:], -8.0)
        nc.vector.tensor_copy(D_Bpad[:, 0:85], D_B4[:])
        D_Bp = ap_.tile([128, NB], f32, tag="D_Bp")
        for gi in range(4):
            nc.vector.tensor_copy(D_Bp[gi * 32:(gi + 1) * 32, :],
                                  D_Bpad[gi * 32:(gi + 1) * 32, gi:88:4])

        # ---- R loops ----
        # Even iterations (ACT): tmp = relu(P + d_i) with fused S-column accum.
        # Odd iterations (DVE tensor_tensor_reduce): tmp' = max(P, -d_i)
        #   = relu(P + d_i) - d_i, with fused accum S'[:,i] = sum_j tmp'.
        # T psum accumulates tmp/tmp' via identity matmuls; afterwards
        #   T_true[c,j] = T_psum[c,j] + sum_{i odd} d[c,i]   (per-lane bias)
        #   S_eff[c,i] = S_raw[c,i]/979 + (d[c,i] if i odd else 0)
        # both corrections ride existing copy ops (bias add / masked D add).
        # Some odd pairs are folded (tensor_tensor add) on DVE or GpSimd so
        # the PE streams fewer tiles.
        def r_loop(P_t, D_cols, negD, n_iter, s_tile, psl, psh, id_ap, cfg):
            n_ttr = n_iter // 2
            n_pairs = n_ttr // 2
            exs = []  # alternate executors so neither engine gets a burst
            p, dv = cfg["nf_pool"], cfg["nf_dve"]
            while (p or dv) and len(exs) < n_pairs:
                if p:
                    exs.append("pool")
                    p -= 1
                if dv and len(exs) < n_pairs:
                    exs.append("dve")
                    dv -= 1
            folds = [None] * n_pairs
            for k, ex in enumerate(exs):  # spread folded pairs evenly
                folds[int((k + 0.5) * n_pairs / len(exs))] = ex
            n_f = sum(1 for f in folds if f)
            ns = n_iter - n_f
            si = 0
            pend = []
            pair_idx = 0

            def stream(t):
                nonlocal si
                nc.tensor.matmul(psl[:], id_ap, t[:, 0:512],
                                 start=(si == 0), stop=(si == ns - 1))
                nc.tensor.matmul(psh[:], id_ap, t[:, 512:LP3],
                                 start=(si == 0), stop=(si == ns - 1))
                si += 1

            for i in range(n_iter):
                tm = tp.tile([128, LP3], bf16, tag="rtmp")
                if i % 2 == 0:
                    nc.scalar.activation(tm[:], P_t[:], AF.Relu,
                                         bias=D_cols[:, i:i + 1],
                                         accum_out=s_tile[:, i:i + 1])
                    stream(tm)
                    continue
                if R_ODD_OP == "ttr":
                    nc.vector.tensor_tensor_reduce(
                        out=tm[:], in0=P_t[:],
                        in1=negD[:, i:i + 1].broadcast_to((128, LP3)),
                        scale=1.0, scalar=0.0, op0=ALU.max, op1=ALU.add,
                        accum_out=s_tile[:, i:i + 1])
                else:
                    # same shifted form via scalar_tensor_tensor:
                    # max(P, -d) + 0, fused row-sum accum
                    nc.vector.scalar_tensor_tensor(
                        out=tm[:], in0=P_t[:], scalar=negD[:, i:i + 1],
                        in1=zeros_t[:], op0=ALU.max, op1=ALU.add,
                        accum_out=s_tile[:, i:i + 1])
                pend.append(tm)
                if len(pend) == 2:
                    ex = folds[pair_idx] if pair_idx < n_pairs else None
                    pair_idx += 1
                    if ex:
                        fo = fp.tile([128, LP3], bf16, tag="rfold")
                        eng = nc.vector if ex == "dve" else nc.gpsimd
                        eng.tensor_tensor(out=fo[:], in0=pend[0][:],
                                          in1=pend[1][:], op=ALU.add)
                        stream(fo)
                    else:
                        stream(pend[0])
                        stream(pend[1])
                    pend = []
            for t in pend:
                stream(t)
            assert si == ns, (si, ns)

        # negated D columns (shifted-form scalar) and odd-masked D (corrections)
        def neg_mask(D_cols, L, tag, mask_ap):
            nD = ap_.tile([128, L], f32, tag=f"nD_{tag}")
            nc.vector.tensor_scalar(out=nD[:], in0=D_cols[:], scalar1=-1.0,
                                    scalar2=None, op0=ALU.mult)
            Dm = ap_.tile([128, L], f32, tag=f"Dm_{tag}")
            nc.vector.tensor_tensor(out=Dm[:], in0=D_cols[:], in1=mask_ap,
                                    op=ALU.mult)
            return nD, Dm

        negD_A, Dm_A = neg_mask(D_A, LD3, "A", g("blob_a", "dmaskA"))
        negD_B, Dm_B = neg_mask(D_Bp, NB, "B", g("blob_a", "dmaskB"))
        if R_ODD_OP == "stt":
            zeros_t = ap_.tile([128, LP3], bf16, tag="zeros")
            nc.vector.memset(zeros_t[:], 0.0)

        S_A = ap_.tile([128, LD3], f32, tag="S_A")
        TA0 = pT.tile([128, 512], f32, tag="TA0")
        TA1 = pT.tile([128, LP3 - 512], f32, tag="TA1")
        r_loop(P_A, D_A, negD_A, LD3, S_A, TA0, TA1, g("blob_a", "id128"), RCFG_A)

        # FC weights DMA, gated on the R loop having started (S_A col 0 is
        # written by iteration 0): the copy below creates the dependency.
        nc.gpsimd.tensor_copy(nc_blob["blob_q"][0:1, 0:1], S_A[0:1, 0:1])
        nc.sync.dma_start(out=nc_blob["blob_q"][:], in_=blob_d["blob_q"][:])

        S_B4 = ap_.tile([128, NB], f32, tag="S_B4")
        TB0 = pT.tile([32, 512], f32, tag="TB0")
        TB1 = pT.tile([32, LP3 - 512], f32, tag="TB1")
        r_loop(P_B4, D_Bp, negD_B, NB, S_B4, TB0, TB1, g("blob_a", "id4"), RCFG_B)

        # T bias corrections: dsum[c] = sum_{i odd} d[c,i]
        dsA = ap_.tile([128, 1], f32, tag="dsA")
        nc.vector.reduce_sum(dsA[:], Dm_A[:], axis=AX.X)
        dsB4f = ap_.tile([128, 1], f32, tag="dsB4f")
        nc.vector.reduce_sum(dsB4f[:], Dm_B[:], axis=AX.X)
        dsB4 = ap_.tile([128, 1], bf16, tag="dsB4")
        nc.vector.tensor_copy(dsB4[:], dsB4f[:])
        psds = pp.tile([32, 1], f32, tag="ps")
        nc.tensor.matmul(psds[:], g("blob_a", "id4"), dsB4[:], start=True, stop=True)
        dsB = ap_.tile([32, 1], f32, tag="dsB")
        nc.vector.tensor_copy(dsB[:], psds[:])

        # S_eff = S_raw/979 + masked D  -> bf16 rhs tiles (atte ca uses scale 1)
        S_Ab = ap_.tile([128, LD3], bf16, tag="S_Ab")
        nc.vector.scalar_tensor_tensor(out=S_Ab[:], in0=S_A[:], scalar=1.0 / LP3,
                                       op0=ALU.mult, in1=Dm_A[:], op1=ALU.add)
        S_B4e = ap_.tile([128, NB], bf16, tag="S_B4e")
        nc.vector.scalar_tensor_tensor(out=S_B4e[:], in0=S_B4[:], scalar=1.0 / LP3,
                                       op0=ALU.mult, in1=Dm_B[:], op1=ALU.add)
        S_Bb = ap_.tile([32, LD3], bf16, tag="S_Bb")
        for gi in range(4):
            cnt = NB if gi == 0 else NB - 1
            nc.vector.tensor_copy(S_Bb[:, gi:gi + 4 * (cnt - 1) + 1:4],
                                  S_B4e[gi * 32:(gi + 1) * 32, 0:cnt])
        # T psum -> bf16 sbuf with the dsum bias (split across ACT and DVE)
        T_Ab = ap_.tile([128, LP3], bf16, tag="T_Ab")
        nc.scalar.activation(T_Ab[:, 0:512], TA0[:], AF.Identity, bias=dsA[:])
        nc.vector.tensor_scalar(out=T_Ab[:, 512:LP3], in0=TA1[:], scalar1=dsA[:],
                                scalar2=None, op0=ALU.add)
        T_Bb = ap_.tile([32, LP3], bf16, tag="T_Bb")
        nc.scalar.activation(T_Bb[:, 0:512], TB0[:], AF.Identity, bias=dsB[:])
        nc.vector.tensor_scalar(out=T_Bb[:, 512:LP3], in0=TB1[:], scalar1=dsB[:],
                                scalar2=None, op0=ALU.add)
        S_ch = [S_Ab, S_Bb]
        T_ch = [T_Ab, T_Bb]

        # ---- attention outputs: sigmoid((sum/n) @ att_w + att_b) ----
        def atte(rhs_ch, L, scale, tag):
            res = []
            for which, (o, w) in enumerate(CH):
                y = ap_.tile([w, L], bf16, tag=f"{tag}{which}")
                for l0 in range(0, L, 512):
                    cs = min(512, L - l0)
                    ps = pp.tile([w, 512], f32, tag="ps")
                    for j in range(2):
                        nc.tensor.matmul(ps[:, :cs],
                                         g("blob_a", f"aw{'AB'[j]}", cs=(o, w)),
                                         rhs_ch[j][:, l0:l0 + cs],
                                         start=(j == 0), stop=(j == 1))
                    nc.scalar.activation(y[:, l0:l0 + cs], ps[:, :cs], AF.Sigmoid,
                                         bias=g("blob_f", f"ab{'AB'[which]}"),
                                         scale=scale)
                res.append(y)
            return res

        ca = atte(S_ch, LD3, 1.0, "ca")  # S_eff already divided by LP3
        pa = atte(T_ch, LP3, 1.0 / LD3, "pa")

        # ---- gate + global max pool: v = max_l(src * (0.5 + atte)) ----
        # gt = atte + 0.5 on ACT (idle here); m and the max-reduce on DVE
        vecs = {}
        for (src, att_, L, tag) in [(dc, ca, LD3, "d"), (pc, pa, LP3, "p")]:
            for which, (o, w) in enumerate(CH):
                gt = tp.tile([w, L], bf16, tag=f"g_{tag}{which}")
                nc.scalar.activation(gt[:], att_[which][:], AF.Identity,
                                     bias=g("blob_f", "half", rs=(0, w)))
                m = tp.tile([w, L], bf16, tag=f"m_{tag}{which}")
                nc.vector.tensor_tensor(out=m[:], in0=src[which][:], in1=gt[:],
                                        op=ALU.mult)
                v = ap_.tile([w, 1], bf16, tag=f"v_{tag}{which}")
                nc.vector.reduce_max(v[:], m[:], axis=AX.X)
                vecs[f"{tag}{which}"] = v
        # pair layout: [dvecA(128), dvecB(32), pvecA(128), pvecB(32)]
        vlist = [vecs["d0"], vecs["d1"], vecs["p0"], vecs["p1"]]

        # ---- FC head (bf16 weight-stationary) ----
        def lrelu_bias(ps, b_ap, ncols, tag):
            h = ap_.tile([128, ncols], f32, tag=f"h_{tag}")
            nc.vector.tensor_tensor(out=h[:], in0=ps[:, :ncols], in1=b_ap,
                                    op=ALU.add)
            t1 = tp.tile([128, ncols], f32, tag="fct")
            nc.vector.tensor_scalar(out=t1[:], in0=h[:], scalar1=0.01,
                                    scalar2=None, op0=ALU.mult)
            h2 = ap_.tile([128, ncols], bf16, tag=f"h2_{tag}")
            nc.vector.tensor_tensor(out=h2[:], in0=h[:], in1=t1[:], op=ALU.max)
            return h2

        ps1 = pp.tile([128, 8], f32, tag="ps")
        for oc in range(8):
            for gi in range(4):
                nc.tensor.matmul(ps1[:, oc:oc + 1],
                                 g("blob_q", f"fc1w_{gi}", cs=(oc * 128, 128)),
                                 vlist[gi][:], start=(gi == 0), stop=(gi == 3))
        h1 = lrelu_bias(ps1, g("blob_f", "fc1b"), 8, "1")

        ps2 = pp.tile([128, 8], f32, tag="ps")
        for oc in range(8):
            for gi in range(8):
                nc.tensor.matmul(ps2[:, oc:oc + 1],
                                 g("blob_q", f"fc2w_{gi}", cs=(oc * 128, 128)),
                                 h1[:, gi:gi + 1], start=(gi == 0), stop=(gi == 7))
        h2 = lrelu_bias(ps2, g("blob_f", "fc2b"), 8, "2")

        ps3 = pp.tile([128, 4], f32, tag="ps")
        for oc in range(4):
            for gi in range(8):
                nc.tensor.matmul(ps3[:, oc:oc + 1],
                                 g("blob_q", f"fc3w_{gi}", cs=(oc * 128, 128)),
                                 h2[:, gi:gi + 1], start=(gi == 0), stop=(gi == 7))
        h3 = lrelu_bias(ps3, g("blob_f", "fc3b"), 4, "3")

        pso = pp.tile([2, 1], f32, tag="ps")
        for gi in range(4):
            nc.tensor.matmul(pso[:], g("blob_a", f"outw_{gi}"), h3[:, gi:gi + 1],
                             start=(gi == 0), stop=(gi == 3))
        ob = ap_.tile([2, 1], f32, tag="ob")
        nc.scalar.activation(ob[:], pso[:], AF.Identity, bias=g("blob_f", "outb"))
        nc.sync.dma_start(out=out_d[:], in_=ob[:])

    nc.compile()
    return nc


def _prep_inputs(inputs):
    """Host-side layout prep. Returns (shared_params, per_core_fn)."""
    import ml_dtypes
    bf = ml_dtypes.bfloat16
    a = lambda x: np.asarray(x)
    rep4 = lambda x: np.tile(x, (4, 1))

    fills = {}
    fills["ones"] = np.ones((1, 128), np.float32)
    fills["embd"] = a(inputs["drug_emb"])
    fills["embp"] = a(inputs["prot_emb"])
    for pre, w1, w2, w3, k2s, k3 in [("d", "dw1", "dw2", "dw3", 3, 8),
                                     ("p", "pw1", "pw2", "pw3", 4, 12)]:
        t1 = a(inputs[w1]).transpose(2, 1, 0)
        t2 = a(inputs[w2]).transpose(2, 1, 0)
        t3 = a(inputs[w3]).transpose(2, 1, 0)
        z24 = np.zeros((24, 2 * CONV), np.float32)
        for k in range(2):
            fills[f"{pre}w1s_{k}"] = np.vstack([t1[2 * k], t1[2 * k + 1]])
        for k in range(k2s):
            fills[f"{pre}w2s_{k}"] = np.vstack([t2[2 * k], z24, t2[2 * k + 1]])
        for k in range(k3):
            fills[f"{pre}w3_{k}"] = t3[k]
    fills["id128"] = np.eye(128, dtype=np.float32)
    fills["id4"] = np.tile(np.eye(32, dtype=np.float32), (4, 1))
    mA = np.zeros((128, LD3), np.float32)
    mA[:, 1::2] = 1.0
    mB = np.zeros((128, NB), np.float32)
    mB[:, 1::2] = 1.0
    fills["dmaskA"], fills["dmaskB"] = mA, mB
    for key, wname in [("daw", "d_att_w"), ("paw", "p_att_w"), ("aw", "att_w")]:
        w = a(inputs[wname])
        fills[f"{key}A"] = w[0:128]
        fills[f"{key}B"] = w[128:160]
    for key, wname in [("dawr", "d_att_w"), ("pawr", "p_att_w")]:
        w = np.tile(a(inputs[wname])[:, 128:160], (1, 4))
        fills[f"{key}A"] = w[0:128]
        fills[f"{key}B"] = w[128:160]
    for i, (o, w) in enumerate(FCCH):
        fills[f"fc1w_{i}"] = a(inputs["fc1_w"])[o:o + w]
    for gi in range(8):
        fills[f"fc2w_{gi}"] = a(inputs["fc2_w"])[gi * 128:(gi + 1) * 128]
        fills[f"fc3w_{gi}"] = a(inputs["fc3_w"])[gi * 128:(gi + 1) * 128]
    for gi in range(4):
        fills[f"outw_{gi}"] = a(inputs["out_w"])[gi * 128:(gi + 1) * 128]
    # f32 blob
    fills["half"] = np.full((128, 1), 0.5, np.float32)
    fills["iota"] = np.arange(128, dtype=np.float32).reshape(128, 1)
    for key, bname in [("db1", "db1"), ("db2", "db2"), ("pb1", "pb1"),
                       ("pb2", "pb2")]:
        fills[key] = a(inputs[bname]).reshape(-1, 1)
    for key, bname in [("db3", "db3"), ("pb3", "pb3"), ("dab", "d_att_b"),
                       ("pab", "p_att_b"), ("ab", "att_b")]:
        v = a(inputs[bname]).reshape(-1, 1)
        fills[f"{key}A"] = v[0:128]
        fills[f"{key}B"] = v[128:160]
    fills["dabr"] = rep4(a(inputs["d_att_b"]).reshape(-1, 1)[128:160])
    fills["pabr"] = rep4(a(inputs["p_att_b"]).reshape(-1, 1)[128:160])
    fills["fc1b"] = a(inputs["fc1_b"]).reshape(8, 128).T
    fills["fc2b"] = a(inputs["fc2_b"]).reshape(8, 128).T
    fills["fc3b"] = a(inputs["fc3_b"]).reshape(4, 128).T
    fills["outb"] = a(inputs["out_b"]).reshape(2, 1)

    bdt = {"blob_f": np.float32}
    shared = {}
    for bname, pos, w in BLOBS:
        dt = bdt.get(bname, bf)
        arr = np.zeros((128, w), dt)
        for name, (r, c0, cw) in pos.items():
            arr[0:r, c0:c0 + cw] = fills[name].astype(dt)
        shared[bname] = arr

    drug = a(inputs["drug"]).astype(bf)
    prot = a(inputs["protein"]).astype(bf)

    def per_core(i):
        m = dict(shared)
        m["drug_idx"] = np.ascontiguousarray(drug[i:i + 1])
        m["prot_idx"] = np.ascontiguousarray(prot[i:i + 1])
        return m

    return shared, per_core


def kernel(**inputs):
    from concourse.bass_utils import run_bass_kernel_spmd

    if "nc" not in _CACHE:
        _CACHE["nc"] = _build()
    nc = _CACHE["nc"]
    _, per_core = _prep_inputs(inputs)
    in_maps = [per_core(i) for i in range(B)]
    r = run_bass_kernel_spmd(nc, in_maps, core_ids=list(range(B)))
    out = np.stack([r.results[i]["out"].reshape(2) for i in range(B)])
    return out.astype(np.float32)
